# revision 1
# baseline (speedup 1.0000x reference)
"""Trainium2 Bass kernel for nn_DecoderLSTM.

Key observation: the reference module never reads `features` — the LSTM input
starts at zeros and is fed back from the predicted point, and h/c start at
zeros.  Every batch row therefore computes the *identical* trajectory
p[t] (t=0..83); the per-row output is just p[t] masked by t < seq_lengths[b].

So the kernel computes the single 84-step two-layer LSTM trajectory on each
NeuronCore (redundantly, no cross-core communication), then broadcasts it
across the 128-partition batch tiles with a per-row length mask and streams
the masked tiles to DRAM.  Batch dim is sharded across the 8 cores.

Layouts (per core):
  - state s_t: [128, 9] fp16; cols 0:4 = h0, 4:8 = h1, col 8 = x (partitions 0:3)
  - c0, c1:    [128, 4] fp32 (in-place update)
  - gates:     PSUM [128, 16] fp32; gate dim d = m*128 + p, gates reordered
               host-side to (i, f, o, g) so cols 0:12 take sigmoid, 12:16 tanh
  - weights:   lhsT tiles [K=128, M=128] fp16, free index = k*2048 + m*128 + j
  - trajectory history: PSUM row [1, 252] fp32 accumulated via M=1 matmuls
"""

import os
import numpy as np

B = 16384
H = 512
T = 84
IN = 3
N_CORES = 8
NB = B // N_CORES          # 2048 rows per core
M_TILES = 16               # 2048 gate dims / 128
BT = NB // 128             # 16 batch tiles per core
F_OUT = T * IN             # 252

_COMPILED = None           # (nc, names) cache
LAST_RESULTS = None        # BassKernelResults from the last run (for test.py)


def _gate_reorder(a, axis=0):
    """torch gate order (i,f,g,o) -> (i,f,o,g) along `axis` (size 4H)."""
    parts = np.split(a, 4, axis=axis)
    return np.concatenate([parts[0], parts[1], parts[3], parts[2]], axis=axis)


def _lhsT_tiles(wT, kt):
    """wT: [K, 2048] -> [128, kt*16*128] with free index (k, m, j)."""
    K = wT.shape[0]
    assert K == kt * 128
    a = wT.reshape(kt, 128, M_TILES, 128)       # [k, p, m, j]
    return np.ascontiguousarray(a.transpose(1, 0, 2, 3).reshape(128, kt * 2048))


def _build_program():
    import concourse.bass as bass
    import concourse.tile as tile
    import concourse.mybir as mybir
    from contextlib import ExitStack

    f16 = mybir.dt.float16
    f32 = mybir.dt.float32
    AF = mybir.ActivationFunctionType
    Alu = mybir.AluOpType

    class SplitDrainTileContext(tile.TileContext):
        """This container's walrus allows only one sync-wait per instruction;
        Tile's kernel-tail drain carries one wait per live semaphore.  Split
        it into a chain of single-wait drains (same semantics: by the last
        drain every semaphore has reached its target)."""

        def _drain_and_barrier(self, tick_clock, wait_clock):
            from concourse.vector_clock import ScopedClock
            drain_inst = self.nc.sync.drain()
            wait_clock.add_sem_waits(
                drain_inst.ins, ScopedClock({None: tick_clock.global_clock}))
            si = drain_inst.ins.sync_info
            waits = list(si.on_wait or []) if si is not None else []
            if len(waits) > 1:
                ups = list(si.on_update or [])
                drain_inst.ins.sync_info = mybir.SyncInfo(
                    on_wait=[waits[0]], on_update=ups)
                for w in waits[1:]:
                    d2 = self.nc.sync.drain()
                    d2.ins.sync_info = mybir.SyncInfo(on_wait=[w], on_update=[])
            self.nc.all_engine_barrier()
            popped = self.nc._tile_sem_poison_stack.pop()
            assert popped is self._sem_poison
            self.nc.clear_and_free_semaphores(list(self.sems.allocated().values()))
            self.nc.all_engine_barrier()

    nc = bass.Bass()

    w0T = nc.declare_dram_parameter("w0T", [128, 4 * 2048], f16, isOutput=False)
    w1T = nc.declare_dram_parameter("w1T", [128, 8 * 2048], f16, isOutput=False)
    wxT = nc.declare_dram_parameter("wxT", [3, 2048], f16, isOutput=False)
    wpT = nc.declare_dram_parameter("wpT", [128, 12], f16, isOutput=False)
    b0d = nc.declare_dram_parameter("b0", [128, 16], f32, isOutput=False)
    b1d = nc.declare_dram_parameter("b1", [128, 16], f32, isOutput=False)
    bpd = nc.declare_dram_parameter("bp", [3, 1], f32, isOutput=False)
    bprepd = nc.declare_dram_parameter("bprep", [1, F_OUT], f32, isOutput=False)
    tvalsd = nc.declare_dram_parameter("tvals", [1, F_OUT], f32, isOutput=False)
    lensd = nc.declare_dram_parameter("lens", [NB], f32, isOutput=False)
    outd = nc.declare_dram_parameter("out", [NB, F_OUT], f32, isOutput=True)

    with ExitStack() as ctx:
        tc = ctx.enter_context(SplitDrainTileContext(nc))
        const = ctx.enter_context(tc.tile_pool(name="const", bufs=1))
        states = ctx.enter_context(tc.tile_pool(name="states", bufs=4))
        tmp = ctx.enter_context(tc.tile_pool(name="tmp", bufs=12))
        outp = ctx.enter_context(tc.tile_pool(name="outp", bufs=1))
        # persistent PSUM tensors (no pool releases -> same-engine WAW needs
        # no semaphores; every matmul then carries at most one sync wait)
        bankA = ctx.enter_context(nc.psum_tensor([128, max(32, 2 * F_OUT)], f32))
        bankB = ctx.enter_context(nc.psum_tensor([128, 32], f32))
        bankC = ctx.enter_context(nc.psum_tensor([4, F_OUT + 1], f32))

        # ---- constants / weights into SBUF ----
        w0s = const.tile([128, 4 * 2048], f16)
        nc.sync.dma_start(w0s[:], w0T[:, :])
        w1s = const.tile([128, 8 * 2048], f16)
        nc.sync.dma_start(w1s[:, 0:4 * 2048], w1T[:, 0:4 * 2048])
        nc.sync.dma_start(w1s[:, 4 * 2048:], w1T[:, 4 * 2048:])
        wxs = const.tile([3, 2048], f16)
        nc.sync.dma_start(wxs[:], wxT[:, :])
        wps = const.tile([128, 12], f16)
        nc.sync.dma_start(wps[:], wpT[:, :])
        b0s = const.tile([128, 16], f32)
        nc.sync.dma_start(b0s[:], b0d[:, :])
        b1s = const.tile([128, 16], f32)
        nc.sync.dma_start(b1s[:], b1d[:, :])
        bps = const.tile([3, 1], f32)
        nc.sync.dma_start(bps[:], bpd[:, :])
        bpreps = const.tile([1, F_OUT], f32)
        nc.sync.dma_start(bpreps[:], bprepd[:, :])
        tvalss = const.tile([1, F_OUT], f32)
        nc.sync.dma_start(tvalss[:], tvalsd[:, :])
        lenss = const.tile([128, BT], f32)
        nc.sync.dma_start(lenss[:], lensd.rearrange("(m p) -> p m", p=128))
        ones1 = const.tile([1, 128], f32)
        nc.vector.memset(ones1[:], 1.0)

        c0 = const.tile([128, 4], f32)
        c1 = const.tile([128, 4], f32)

        prow = bankC[0:1, 0:F_OUT]           # trajectory history, PSUM resident

        # Sync-wait absorbers: walrus allows only one sync-wait per compute
        # instruction, so drain each const-DMA semaphore into the DVE / PE
        # vector clocks here, before any compute pairs it with another wait.
        absb = const.tile([1, 6], f32)
        nc.vector.tensor_copy(absb[:, 0:1], b0s[0:1, 0:1])
        nc.vector.tensor_copy(absb[:, 1:2], b1s[0:1, 0:1])
        nc.vector.tensor_copy(absb[:, 2:3], bps[0:1, 0:1])
        nc.vector.tensor_copy(absb[:, 3:4], bpreps[0:1, 0:1])
        nc.vector.tensor_copy(absb[:, 4:5], tvalss[0:1, 0:1])
        nc.vector.tensor_copy(absb[:, 5:6], lenss[0:1, 0:1])
        nc.tensor.ldweights(w1s[:, 0:128])
        nc.tensor.ldweights(wxs[:, 0:128])
        nc.tensor.ldweights(wps[:, 0:3])

        cell_no = [0]
        def lstm_cell(gb_getter, c_sb, h_out_ap, first):
            """Biased gates (i,f,o,g layout) -> update c, write h'."""
            u = cell_no[0]; cell_no[0] += 1
            gb = gb_getter()
            sg = tmp.tile([128, 16], f32, tag=f"sg{u}", bufs=1)
            nc.scalar.activation(sg[:], gb[:], AF.Sigmoid)
            tg = tmp.tile([128, 4], f32, tag=f"tg{u}", bufs=1)
            nc.vector.tensor_scalar(tg[:], sg[:, 12:16], 2.0, -1.0,
                                    Alu.mult, Alu.add)  # tanh(g)=2*sig(2g)-1
            t1 = tmp.tile([128, 4], f32, tag="t1")
            nc.vector.tensor_mul(t1[:], sg[:, 0:4], tg[:])      # sig(i)*tanh(g)
            if first:
                nc.vector.tensor_copy(c_sb[:], t1[:])           # c was zero
            else:
                t2 = tmp.tile([128, 4], f32, tag="t2")
                nc.vector.tensor_mul(t2[:], sg[:, 4:8], c_sb[:])  # sig(f)*c
                nc.vector.tensor_add(c_sb[:], t1[:], t2[:])       # c' in place
            tcn = tmp.tile([128, 4], f32, tag=f"tc{u}", bufs=1)
            nc.scalar.activation(tcn[:], c_sb[:], AF.Tanh)
            nc.vector.tensor_mul(h_out_ap, sg[:, 8:12], tcn[:])  # sig(o)*tanh(c')

        def emit_head(s_t, t):
            """head for step t: p = W_pc @ h1'(t) + b_pc -> x feedback + history."""
            pcol = bankC[0:3, F_OUT:F_OUT + 1]
            for k in range(4):
                nc.tensor.matmul(
                    pcol,
                    lhsT=wps[:, 3 * k:3 * k + 3],
                    rhs=s_t[:, 4 + k:5 + k],
                    start=(k == 0), stop=(k == 3),
                )
            for k in range(4):
                nc.tensor.matmul(
                    prow[0:1, 3 * t:3 * t + 3],
                    lhsT=s_t[:, 4 + k:5 + k],
                    rhs=wps[:, 3 * k:3 * k + 3],
                    start=(k == 0), stop=(k == 3),
                )
            nc.vector.tensor_add(s_t[0:3, 8:9], pcol, bps[:])

        # PE queue is in-order, so emission order = PE execution order.  Per
        # iteration t: (1) cell0 h-passes (ready since chain0(t-1); they hide
        # chain1(t-1)), (2) head(t-1) (h1'(t-1) ready by now), (3) x-passes,
        # (4) cell0 elementwise, (5) cell1 W_hh1 passes (hide cell0's
        # elementwise chain), (6) cell1 W_ih1 passes, (7) cell1 elementwise.
        # Each PSUM column accumulation group is contiguous; the four gate
        # contributions go to separate PSUM regions summed by the DVE.
        s_prev = None
        for t in range(T):
            s_new = states.tile([128, 9], f16, tag="s")

            # ---- cell 0: gates0 = W_hh0 @ h0 + W_ih0 @ x  (zero at t=0) ----
            if t == 0:
                lstm_cell(lambda: b0s, c0, s_new[:, 0:4], True)
            else:
                if t == 1:
                    # absorb the remaining weight-DMA semaphores now, after
                    # the t=0 matmuls had a chance to run
                    nc.tensor.ldweights(w0s[:, 0:128])
                    nc.tensor.ldweights(w1s[:, 4 * 2048:4 * 2048 + 128])
                g0 = bankA[:, 0:16]
                for m in range(M_TILES):
                    for k in range(4):
                        nc.tensor.matmul(
                            g0[:, m:m + 1],
                            lhsT=w0s[:, k * 2048 + m * 128:k * 2048 + (m + 1) * 128],
                            rhs=s_prev[:, k:k + 1],
                            start=(k == 0), stop=(k == 3),
                        )
                emit_head(s_prev, t - 1)
                xg = bankA[:, 16:32]
                xg_last = None
                for m in range(M_TILES):
                    xg_last = nc.tensor.matmul(
                        xg[:, m:m + 1],
                        lhsT=wxs[:, m * 128:(m + 1) * 128],
                        rhs=s_prev[0:3, 8:9],
                        start=True, stop=True,
                    )

                def gb0_get():
                    gb = tmp.tile([128, 16], f32, tag="gb")
                    nc.vector.tensor_add(gb[:], g0[:], b0s[:])
                    nc.vector.tensor_add(gb[:], gb[:], xg[:])
                    return gb
                lstm_cell(gb0_get, c0, s_new[:, 0:4], False)

            # ---- cell 1: gates1 = W_hh1 @ h1 + W_ih1 @ h0' ----
            g1a = bankB[:, 0:16]
            g1b = bankB[:, 16:32]
            if t > 0:
                from concourse.tile_rust import add_dep_helper
                for m in range(M_TILES):
                    for k in range(4, 8):
                        mm = nc.tensor.matmul(
                            g1a[:, m:m + 1],
                            lhsT=w1s[:, k * 2048 + m * 128:k * 2048 + (m + 1) * 128],
                            rhs=s_prev[:, k:k + 1],
                            start=(k == 4), stop=(k == 7),
                        )
                        if k == 4 and xg_last is not None:
                            add_dep_helper(mm.ins, xg_last.ins, sync=False,
                                           reason="x-passes feed chain0; run first")
            for m in range(M_TILES):
                for k in range(4):
                    nc.tensor.matmul(
                        g1b[:, m:m + 1],
                        lhsT=w1s[:, k * 2048 + m * 128:k * 2048 + (m + 1) * 128],
                        rhs=s_new[:, k:k + 1],
                        start=(k == 0), stop=(k == 3),
                    )

            def gb1_get():
                gb = tmp.tile([128, 16], f32, tag="gb")
                nc.vector.tensor_add(gb[:], g1b[:], b1s[:])
                if t > 0:
                    nc.vector.tensor_add(gb[:], gb[:], g1a[:])
                return gb
            lstm_cell(gb1_get, c1, s_new[:, 4:8], t == 0)

            s_prev = s_new

        emit_head(s_prev, T - 1)

        # ---- broadcast + mask + store ----
        # one [1, 504] row = [p+b_pc | tvals]; single K=1 matmul broadcasts
        # both across 128 partitions (one PSUM bank: 504 fp32 < 512)
        row2 = const.tile([1, 2 * F_OUT], f32)
        nc.vector.tensor_add(row2[:, 0:F_OUT], prow, bpreps[:])
        nc.vector.tensor_copy(row2[:, F_OUT:2 * F_OUT], tvalss[:])
        bc_ps = bankA[:, 0:2 * F_OUT]
        nc.tensor.matmul(bc_ps, lhsT=ones1[:], rhs=row2[:],
                         start=True, stop=True)
        bc = const.tile([128, 2 * F_OUT], f32)
        nc.scalar.copy(bc[:], bc_ps)
        pbc = bc[:, 0:F_OUT]
        tvbc = bc[:, F_OUT:2 * F_OUT]

        # 16 batch tiles in one SBUF buffer; store in 4 chunked DMAs so the
        # stores overlap the remaining mask computations
        ot = outp.tile([128, BT * F_OUT], f32, tag="ot")
        out_r = outd.rearrange("(n p) f -> p n f", p=128)
        for i in range(BT):
            # out_row = (tvals < len) * p_broadcast, fused in one DVE op
            nc.vector.scalar_tensor_tensor(
                ot[:, i * F_OUT:(i + 1) * F_OUT], tvbc, lenss[:, i:i + 1],
                pbc, Alu.is_lt, Alu.mult)
            if i % 4 == 3:
                nc.gpsimd.dma_start(
                    out_r[:, i - 3:i + 1, :],
                    ot[:, (i - 3) * F_OUT:(i + 1) * F_OUT])

    return nc


def _dbl_g(a):
    # tanh(g) is computed as 2*sigmoid(2g)-1; fold the 2x into the g rows
    a = a.copy()
    a[3 * 512:] *= 2.0
    return a


def _prep_inputs(inputs):
    f = lambda k: np.asarray(inputs[k], np.float32)
    Wih0 = _dbl_g(_gate_reorder(f("W_ih0")))
    Whh0 = _dbl_g(_gate_reorder(f("W_hh0")))
    Wih1 = _dbl_g(_gate_reorder(f("W_ih1")))
    Whh1 = _dbl_g(_gate_reorder(f("W_hh1")))
    b0 = _dbl_g(_gate_reorder(f("b_ih0") + f("b_hh0")))
    b1 = _dbl_g(_gate_reorder(f("b_ih1") + f("b_hh1")))
    Wpc = f("W_pc")
    bpc = f("b_pc")

    common = {
        "w0T": _lhsT_tiles(Whh0.T.copy(), 4).astype(np.float16),
        "w1T": _lhsT_tiles(np.concatenate([Wih1.T, Whh1.T], 0), 8).astype(np.float16),
        "wxT": np.ascontiguousarray(Wih0.T).astype(np.float16),
        "wpT": np.ascontiguousarray(
            Wpc.T.reshape(4, 128, 3).transpose(1, 0, 2).reshape(128, 12)
        ).astype(np.float16),
        "b0": np.ascontiguousarray(b0.reshape(16, 128).T),
        "b1": np.ascontiguousarray(b1.reshape(16, 128).T),
        "bp": bpc.reshape(3, 1).copy(),
        "bprep": np.tile(bpc, T).reshape(1, F_OUT).copy(),
        "tvals": np.repeat(np.arange(T, dtype=np.float32), IN).reshape(1, F_OUT),
    }
    lens = np.asarray(inputs["seq_lengths"]).astype(np.float32)
    in_maps = []
    for c in range(N_CORES):
        m = dict(common)
        m["lens"] = np.ascontiguousarray(lens[c * NB:(c + 1) * NB])
        in_maps.append(m)
    return in_maps


def kernel(**inputs):
    global _COMPILED, LAST_RESULTS
    from concourse.bass_utils import run_bass_kernel_spmd

    if _COMPILED is None:
        _COMPILED = _build_program()
    nc = _COMPILED

    in_maps = _prep_inputs(inputs)
    res = run_bass_kernel_spmd(nc, in_maps, list(range(N_CORES)))
    LAST_RESULTS = res
    out = np.concatenate([res.results[c]["out"] for c in range(N_CORES)], axis=0)
    return np.ascontiguousarray(out.reshape(B, T, IN))



# revision 20
# speedup vs baseline: 3.6914x; 3.6914x over previous
"""Trainium2 Bass kernel for nn_DecoderLSTM.

Key observation: the reference module never reads `features` -- the LSTM input
starts at zeros and is fed back from the predicted point, and h/c start at
zeros.  Every batch row therefore computes the *identical* trajectory
p[t] (t=0..83); the per-row output is just p[t] masked by t < seq_lengths[b].

This version replaces the sequential 84-step scan with a parallel-in-time
Gauss-Seidel iteration: all 84 timesteps are updated simultaneously (matmuls
with N=84 moving columns), and the linear cell-state recurrence
c_t = sig(f_t)*c_{t-1} + sig(i_t)*tanh(g_t) is solved exactly within each
sweep by the DVE's tensor_tensor_scan.  Convergence (verified on the host
oracle): rel err 2.3e-3 after 4 sweeps, 8.8e-4 after 5, 2.0e-4 after 6.
The sequential version pays ~40ns of PE weight-load per 128x128 tile for
every one of 84 steps (216 tiles/step -> ~700us); here each weight tile is
loaded once per sweep and serves all 84 columns, so the whole trajectory
costs J_SWEEPS (=6) weight passes instead of 84.

Layouts (per core):
  - states H0s/H1s (ping-pong pair): [128, 4, 85] fp16; col tau holds
    h(tau-1), col 0 is the t=-1 zero state.  chunk k on dim1: h[128k+p].
  - x feed Xs: [4, 85] fp16; rows 0:3 = x (col tau = p_{tau-1}), row 3 = 1
    (carries the cell-0 bias through the x matmul, K=4).
  - gates PSUM G0/G1: [128, 4, 512] fp32 (4 banks; bank q = gate q of
    (i,f,o,g) after host-side reorder; region ml*84 inside the bank is
    h-chunk ml).  g-gate rows are pre-doubled so tanh(g) = 2*sig(2g)-1.
  - weights: lhsT tiles [K=128, M=128] fp16, free index = k*2048 + m*128 + j.
  - head P: [3, 84] fp32 in G0 bank 3, offsets 336:420.
"""

import numpy as np

B = 16384
H = 512
T = 84
IN = 3
N_CORES = 8
NB = B // N_CORES          # 2048 rows per core
M_TILES = 16               # 2048 gate dims / 128
BT = NB // 128             # 16 batch tiles per core
F_OUT = T * IN             # 252
J_SWEEPS = 6

_COMPILED = None           # nc cache
LAST_RESULTS = None        # BassKernelResults from the last run (for test.py)


def _gate_reorder(a, axis=0):
    """torch gate order (i,f,g,o) -> (i,f,o,g) along `axis` (size 4H)."""
    parts = np.split(a, 4, axis=axis)
    return np.concatenate([parts[0], parts[1], parts[3], parts[2]], axis=axis)


def _lhsT_tiles(wT, kt):
    """wT: [K, 2048] -> [128, kt*16*128] with free index (k, m, j)."""
    K = wT.shape[0]
    assert K == kt * 128
    a = wT.reshape(kt, 128, M_TILES, 128)       # [k, p, m, j]
    return np.ascontiguousarray(a.transpose(1, 0, 2, 3).reshape(128, kt * 2048))


def _build_program():
    import concourse.bass as bass
    import concourse.tile as tile
    import concourse.mybir as mybir
    from contextlib import ExitStack

    f16 = mybir.dt.float16
    f32 = mybir.dt.float32
    AF = mybir.ActivationFunctionType
    Alu = mybir.AluOpType

    class SplitDrainTileContext(tile.TileContext):
        """This container's walrus allows only one sync-wait per instruction;
        Tile's kernel-tail drain carries one wait per live semaphore.  Split
        it into a chain of single-wait drains (same semantics: by the last
        drain every semaphore has reached its target)."""

        def _drain_and_barrier(self, tick_clock, wait_clock):
            from concourse.vector_clock import ScopedClock
            drain_inst = self.nc.sync.drain()
            wait_clock.add_sem_waits(
                drain_inst.ins, ScopedClock({None: tick_clock.global_clock}))
            si = drain_inst.ins.sync_info
            waits = list(si.on_wait or []) if si is not None else []
            if len(waits) > 1:
                ups = list(si.on_update or [])
                drain_inst.ins.sync_info = mybir.SyncInfo(
                    on_wait=[waits[0]], on_update=ups)
                for w in waits[1:]:
                    d2 = self.nc.sync.drain()
                    d2.ins.sync_info = mybir.SyncInfo(on_wait=[w], on_update=[])
            self.nc.all_engine_barrier()
            popped = self.nc._tile_sem_poison_stack.pop()
            assert popped is self._sem_poison
            self.nc.clear_and_free_semaphores(list(self.sems.allocated().values()))
            self.nc.all_engine_barrier()

    nc = bass.Bass()

    w0T = nc.declare_dram_parameter("w0T", [128, 4 * 2048], f16, isOutput=False)
    w1T = nc.declare_dram_parameter("w1T", [128, 8 * 2048], f16, isOutput=False)
    wx4T = nc.declare_dram_parameter("wx4T", [4, 2048], f16, isOutput=False)
    wpT = nc.declare_dram_parameter("wpT", [128, 12], f16, isOutput=False)
    b1rd = nc.declare_dram_parameter("b1rep", [128, M_TILES * T], f16, isOutput=False)
    oh3d = nc.declare_dram_parameter("oh3", [3, 3 * 128], f16, isOutput=False)
    bpcd = nc.declare_dram_parameter("bpc84", [3, T], f32, isOutput=False)
    tvd = nc.declare_dram_parameter("tvals", [1, F_OUT], f16, isOutput=False)
    xsid = nc.declare_dram_parameter("xsinit", [4, T + 1], f16, isOutput=False)
    lensd = nc.declare_dram_parameter("lens", [NB], f32, isOutput=False)
    outd = nc.declare_dram_parameter("out", [NB, F_OUT], f32, isOutput=True)

    with ExitStack() as ctx:
        tc = ctx.enter_context(SplitDrainTileContext(nc))
        const = ctx.enter_context(tc.tile_pool(name="const", bufs=1))
        tmp = ctx.enter_context(tc.tile_pool(name="tmp", bufs=2))
        # gates: 16 regions of 84 (region r = q*4+ml for gate q, h-chunk ml),
        # packed 6 regions per PSUM bank -> 3 banks per cell.  Tile chains
        # PSUM *readers* with sync edges at tensor granularity, so each PSUM
        # tensor gets exactly one reader instruction per sweep: G0/G1 are read
        # only by the gb copy, PB (head) only by the Xs add, BC (broadcasts)
        # only by init/epilogue copies.
        GA0 = ctx.enter_context(nc.psum_tensor([128, 2, 512], f32))
        GB0 = ctx.enter_context(nc.psum_tensor([128, 1, 512], f32))
        GA1 = ctx.enter_context(nc.psum_tensor([128, 2, 512], f32))
        GB1 = ctx.enter_context(nc.psum_tensor([128, 1, 512], f32))
        PB = ctx.enter_context(nc.psum_tensor([128, 1, 512], f32))
        BC = ctx.enter_context(nc.psum_tensor([128, 1, 512], f32))
        G0 = (GA0, GB0)
        G1 = (GA1, GB1)
        Pap = PB[0:3, 0, 0:T]              # head output

        def greg(G, r):
            """PSUM AP of gate region r (6-per-bank packing, split tensors)."""
            GA, GB = G
            if r < 12:
                return GA[:, r // 6, (r % 6) * T:(r % 6 + 1) * T]
            return GB[:, 0, (r - 12) * T:(r - 11) * T]

        # ---- constants / weights into SBUF ----
        w0s = const.tile([128, 4 * 2048], f16)
        nc.sync.dma_start(w0s[:], w0T[:, :])
        w1s = const.tile([128, 8 * 2048], f16)
        nc.sync.dma_start(w1s[:, 0:4 * 2048], w1T[:, 0:4 * 2048])
        nc.sync.dma_start(w1s[:, 4 * 2048:], w1T[:, 4 * 2048:])
        wx4s = const.tile([4, 2048], f16)
        nc.sync.dma_start(wx4s[:], wx4T[:, :])
        wpss = const.tile([128, 12], f16)
        nc.sync.dma_start(wpss[:], wpT[:, :])
        b1rs = const.tile([128, M_TILES * T], f16)
        nc.sync.dma_start(b1rs[:], b1rd[:, :])
        oh3s = const.tile([3, 3 * 128], f16)
        nc.sync.dma_start(oh3s[:], oh3d[:, :])
        bpcs = const.tile([3, T], f32)
        nc.sync.dma_start(bpcs[:], bpcd[:, :])
        tvs = const.tile([1, F_OUT], f16)
        nc.sync.dma_start(tvs[:], tvd[:, :])
        lenss = const.tile([128, BT], f32)
        nc.sync.dma_start(lenss[:], lensd.rearrange("(m p) -> p m", p=128))

        ones1 = const.tile([1, 128], f16)
        nc.vector.memset(ones1[:], 1.0)

        # states: ping-pong buffers, col 0 = zero state
        H0s = [const.tile([128, 4, T + 1], f16, name=f"h0_{i}") for i in range(2)]
        H1s = [const.tile([128, 4, T + 1], f16, name=f"h1_{i}") for i in range(2)]
        Xs = const.tile([4, T + 1], f16)
        nc.sync.dma_start(Xs[:], xsid[:, :])  # rows 0:3 zero, row 3 = 1 (bias)
        for s in (*H0s, *H1s):
            nc.vector.memset(s[:, :, 0:1], 0.0)

        # Sync-wait absorbers: walrus allows only one sync-wait per compute
        # instruction; drain each DVE-consumed const-DMA semaphore into the
        # DVE clock, and each weight-DMA semaphore into the PE clock.
        absb = const.tile([1, 5], f32)
        nc.vector.tensor_copy(absb[:, 0:1], b1rs[0:1, 0:1])
        nc.vector.tensor_copy(absb[:, 1:2], bpcs[0:1, 0:1])
        nc.vector.tensor_copy(absb[:, 2:3], tvs[0:1, 0:1])
        nc.vector.tensor_copy(absb[:, 3:4], lenss[0:1, 0:1])
        nc.vector.tensor_copy(absb[:, 4:5], Xs[0:1, 0:1])
        nc.tensor.ldweights(wx4s[0:4, 0:1])
        nc.tensor.ldweights(w1s[:, 0:1])

        # broadcast the t-values row across partitions (once)
        tvbc = const.tile([128, F_OUT], f32)
        nc.tensor.matmul(BC[:, 0, F_OUT:2 * F_OUT], lhsT=ones1[:], rhs=tvs[:],
                         start=True, stop=True)
        nc.vector.tensor_copy(tvbc[:], BC[:, 0, F_OUT:2 * F_OUT])

        def cell_elementwise(G, h_out, cellno, j, bias=None):
            """sigmoid gates -> scan for c -> h' = sig(o)*tanh(c).

            walrus allows one sync-wait per instruction, and Tile chains
            PSUM readers with sync edges, so PSUM is written only by the PE
            and read only by the DVE: gates leave PSUM through a DVE
            copy/bias-add and the ACT ops read the SBUF copy.  ACT outputs
            (sg, tg, tcn) get a fresh tile per sweep: reusing them makes the
            ACT-ACT WAW edge emit a second sync-wait.  The g-gate bank (GB)
            is computed first by the PE, so tanh(g) runs while the i/f/o
            matmuls still stream."""
            GA, GB = G
            gbb = tmp.tile([128, 4 * T], f16, tag=f"gbb{cellno}", bufs=2)
            gba = tmp.tile([128, 12 * T], f16, tag=f"gba{cellno}", bufs=2)
            if bias is not None:
                nc.vector.tensor_add(gbb[:], GB[:, 0, 0:4 * T], bias[:, 12 * T:])
            else:
                nc.vector.tensor_copy(gbb[:], GB[:, 0, 0:4 * T])
            tg = tmp.tile([128, 4 * T], f16, tag=f"tg{cellno}_{j}", bufs=1)
            nc.scalar.activation(tg[:], gbb[:], AF.Tanh)       # tanh(g)
            if bias is not None:
                nc.vector.tensor_add(gba[:], GA[:, :, 0:6 * T], bias[:, 0:12 * T])
            else:
                nc.vector.tensor_copy(gba[:], GA[:, :, 0:6 * T])
            sg = tmp.tile([128, 12 * T], f16, tag=f"sg{cellno}_{j}", bufs=1)
            nc.scalar.activation(sg[:], gba[:], AF.Sigmoid)    # sig(i,f,o)
            u = tmp.tile([128, 4 * T], f16, tag=f"u{cellno}", bufs=2)
            nc.vector.tensor_mul(u[:], sg[:, 0:4 * T], tg[:])  # sig(i)*tanh(g)
            cf = tmp.tile([128, 4 * T], f16, tag=f"c{cellno}", bufs=2)
            for c4 in range(4):
                nc.vector.tensor_tensor_scan(
                    cf[:, c4 * T:(c4 + 1) * T],
                    sg[:, 4 * T + c4 * T:4 * T + (c4 + 1) * T],
                    u[:, c4 * T:(c4 + 1) * T],
                    0.0, Alu.mult, Alu.add)   # c_t = sig(f_t)*c_{t-1} + u_t
            tcn = tmp.tile([128, 4 * T], f16, tag=f"tc{cellno}_{j}", bufs=1)
            nc.scalar.activation(tcn[:], cf[:], AF.Tanh)
            nc.vector.tensor_mul(h_out, sg[:, 8 * T:12 * T], tcn[:])

        def emit_head(h1buf):
            """P(:, t) = W_pc @ h1(t), then Xs[:, 1:] = P + b_pc (fp16)."""
            for k in range(4):
                nc.tensor.matmul(Pap, lhsT=wpss[:, 3 * k:3 * k + 3],
                                 rhs=h1buf[:, k, 1:T + 1],
                                 start=(k == 0), stop=(k == 3))
            nc.vector.tensor_add(Xs[0:3, 1:T + 1], Pap, bpcs[:])

        # g-gate regions (12..15, the single GB bank) first: their DVE copy +
        # tanh can start while the i/f/o banks still stream through the PE
        M_ORDER = list(range(12, 16)) + list(range(12))

        for j in range(J_SWEEPS):
            r, w = j % 2, (j + 1) % 2
            if j > 0:
                emit_head(H1s[r])           # head of sweep j-1 -> x for sweep j
            # G0 = W_hh0 @ h0(prev, shifted) + W_ih0 @ x + b0.  Only one PSUM
            # accumulation group may be open per bank, so each region's
            # matmuls (4x W_hh0 + the K=4 x/bias pass) are emitted as one
            # tight group.
            for m in M_ORDER:
                for k in range(4):
                    if j > 0:
                        nc.tensor.matmul(
                            greg(G0, m),
                            lhsT=w0s[:, k * 2048 + m * 128:k * 2048 + (m + 1) * 128],
                            rhs=H0s[r][:, k, 0:T],
                            start=(k == 0), stop=False)
                nc.tensor.matmul(
                    greg(G0, m),
                    lhsT=wx4s[0:4, m * 128:(m + 1) * 128],
                    rhs=Xs[0:4, 0:T],
                    start=(j == 0), stop=True)
            cell_elementwise(G0, H0s[w][:, :, 1:T + 1], 0, j)
            # G1 = W_ih1 @ h0(this sweep) + W_hh1 @ h1(prev, shifted)
            for m in M_ORDER:
                for k in range(4):
                    nc.tensor.matmul(
                        greg(G1, m),
                        lhsT=w1s[:, k * 2048 + m * 128:k * 2048 + (m + 1) * 128],
                        rhs=H0s[w][:, k, 1:T + 1],
                        start=(k == 0), stop=(j == 0 and k == 3))
                if j > 0:
                    for k in range(4, 8):
                        nc.tensor.matmul(
                            greg(G1, m),
                            lhsT=w1s[:, k * 2048 + m * 128:k * 2048 + (m + 1) * 128],
                            rhs=H1s[r][:, k - 4, 0:T],
                            start=False, stop=(k == 7))
            cell_elementwise(G1, H1s[w][:, :, 1:T + 1], 1, j, bias=b1rs[:])

        # final head -> Xs[0:3, 1:85] = final trajectory points (with bias)
        emit_head(H1s[J_SWEEPS % 2])

        # ---- broadcast + mask + store ----
        nc.tensor.ldweights(oh3s[0:3, 0:1])   # absorb oh3 DMA sem into PE
        for i in range(3):
            nc.tensor.matmul(BC[:, 0, i * T:(i + 1) * T],
                             lhsT=oh3s[0:3, i * 128:(i + 1) * 128],
                             rhs=Xs[0:3, 1:T + 1], start=True, stop=True)
        pq = const.tile([128, T, 3], f32)     # interleaved [t, i] -> col 3t+i
        for i in range(3):
            nc.vector.tensor_copy(pq[:, :, i], BC[:, 0, i * T:(i + 1) * T])
        ot = const.tile([128, BT * F_OUT], f32)
        out_r = outd.rearrange("(n p) f -> p n f", p=128)
        for n in range(BT):
            # out_row = (tvals < len) * p_broadcast, fused in one DVE op
            nc.vector.scalar_tensor_tensor(
                ot[:, n * F_OUT:(n + 1) * F_OUT], tvbc[:], lenss[:, n:n + 1],
                pq[:, :, :], Alu.is_lt, Alu.mult)
            if n % 4 == 3:
                nc.gpsimd.dma_start(
                    out_r[:, n - 3:n + 1, :],
                    ot[:, (n - 3) * F_OUT:(n + 1) * F_OUT])

    return nc


def _prep_inputs(inputs):
    f = lambda k: np.asarray(inputs[k], np.float32)
    Wih0 = _gate_reorder(f("W_ih0"))
    Whh0 = _gate_reorder(f("W_hh0"))
    Wih1 = _gate_reorder(f("W_ih1"))
    Whh1 = _gate_reorder(f("W_hh1"))
    b0 = _gate_reorder(f("b_ih0") + f("b_hh0"))
    b1 = _gate_reorder(f("b_ih1") + f("b_hh1"))
    Wpc = f("W_pc")
    bpc = f("b_pc")

    oh3 = np.zeros((3, 3 * 128), np.float16)
    for i in range(3):
        oh3[i, i * 128:(i + 1) * 128] = 1.0

    common = {
        "w0T": _lhsT_tiles(Whh0.T.copy(), 4).astype(np.float16),
        "w1T": _lhsT_tiles(np.concatenate([Wih1.T, Whh1.T], 0), 8).astype(np.float16),
        "wx4T": np.ascontiguousarray(
            np.concatenate([Wih0.T, b0[None, :]], 0)).astype(np.float16),
        "wpT": np.ascontiguousarray(
            Wpc.T.reshape(4, 128, 3).transpose(1, 0, 2).reshape(128, 12)
        ).astype(np.float16),
        "b1rep": np.ascontiguousarray(
            np.repeat(b1.reshape(M_TILES, 128).T[:, :, None], T, axis=2)
            .reshape(128, M_TILES * T)).astype(np.float16),
        "oh3": oh3,
        "bpc84": np.ascontiguousarray(np.repeat(bpc[:, None], T, axis=1)),
        "tvals": np.repeat(np.arange(T, dtype=np.float16), IN).reshape(1, F_OUT),
        "xsinit": np.concatenate(
            [np.zeros((3, T + 1), np.float16), np.ones((1, T + 1), np.float16)], 0),
    }
    lens = np.asarray(inputs["seq_lengths"]).astype(np.float32)
    in_maps = []
    for c in range(N_CORES):
        m = dict(common)
        m["lens"] = np.ascontiguousarray(lens[c * NB:(c + 1) * NB])
        in_maps.append(m)
    return in_maps


def kernel(**inputs):
    global _COMPILED, LAST_RESULTS
    from concourse.bass_utils import run_bass_kernel_spmd

    if _COMPILED is None:
        _COMPILED = _build_program()
    nc = _COMPILED

    in_maps = _prep_inputs(inputs)
    res = run_bass_kernel_spmd(nc, in_maps, list(range(N_CORES)))
    LAST_RESULTS = res
    out = np.concatenate([res.results[c]["out"] for c in range(N_CORES)], axis=0)
    return np.ascontiguousarray(out.reshape(B, T, IN))


# revision 25
# speedup vs baseline: 4.9457x; 1.3398x over previous
"""Trainium2 Bass kernel for nn_DecoderLSTM.

Key observation: the reference module never reads `features` -- the LSTM input
starts at zeros and is fed back from the predicted point, and h/c start at
zeros.  Every batch row therefore computes the *identical* trajectory
p[t] (t=0..83); the per-row output is just p[t] masked by t < seq_lengths[b].

The sequential 84-step scan is replaced by a parallel-in-time Gauss-Seidel
iteration: all 84 timesteps are updated simultaneously (matmuls with N=84
moving columns), and the linear cell-state recurrence
c_t = sig(f_t)*c_{t-1} + sig(i_t)*tanh(g_t) is solved exactly within each
sweep by the DVE's tensor_tensor_scan.  Convergence (verified against the
host oracle): rel err 2.3e-3 after 4 sweeps, 8.8e-4 after 5, 2.0e-4 after 6.
The sequential version pays the PE weight-load for every 128x128 tile at
every one of 84 steps (~40ns/tile, 216 tiles/step -> ~700us); here each
weight tile is loaded once per sweep and serves all 84 columns.

Hardware rules this code is shaped around:
  - walrus allows ONE sync-wait per instruction.  Tile chains PSUM readers
    with sync edges at tensor granularity, so every PSUM tensor has exactly
    one reader instruction per sweep (a DVE copy/bias-add); ACT never reads
    PSUM and every ACT-written tile is fresh per sweep (ACT-ACT WAW edges
    also emit waits).
  - only one PSUM accumulation group may be open per 2KB bank, so each gate
    region's matmuls are emitted as one tight group.
  - gates are grouped into PSUM tensors by gate type, ordered (g,f01 |
    f23,i | o) so the o-gate matmuls stream while the scan/tanh chain runs:
    the PE's post-cell stall is only gb_o -> sig_o -> h'.

Layouts (per core):
  - states H0s/H1s (ping-pong pair): [128, 4, 85] fp16; col tau holds
    h(tau-1), col 0 is the t=-1 zero state; chunk k on dim1: h[128k+p].
  - x feed Xs: [4, 85] fp16; rows 0:3 = x (col tau = p_{tau-1}), row 3 = 1
    (carries the cell-0 bias through the x matmul, K=4).
  - gate region r = q*4 + ml (gate q of (i,f,o,g), h-chunk ml), 84 columns
    each, distributed across three PSUM tensors per cell in SLOT order
    (12,13,14,15,4,5 | 6,7,0,1,2,3 | 8,9,10,11).
  - weights: lhsT tiles [K=128, M=128] fp16, m-major (free = m*512+k*128+j),
    DMA'd in 4 chunks ordered by first use.
"""

import numpy as np

B = 16384
H = 512
T = 84
IN = 3
N_CORES = 8
NB = B // N_CORES          # 2048 rows per core
M_TILES = 16               # 2048 gate dims / 128
BT = NB // 128             # 16 batch tiles per core
F_OUT = T * IN             # 252
J_SWEEPS = 5

# gate regions in emission (slot) order: tensor A = g gates + f chunks 0,1;
# tensor B = f chunks 2,3 + i gates; tensor O = o gates (streamed last)
SLOTS_A = (12, 13, 14, 15, 4, 5)
SLOTS_B = (6, 7, 0, 1, 2, 3)
SLOTS_O = (8, 9, 10, 11)
SLOTS = SLOTS_A + SLOTS_B + SLOTS_O
# weight DMA chunks (4 m-tiles each) in first-use order of SLOTS
WCHUNKS = ((12, 16), (4, 8), (0, 4), (8, 12))

_COMPILED = None           # nc cache
LAST_RESULTS = None        # BassKernelResults from the last run (for test.py)


def _gate_reorder(a, axis=0):
    """torch gate order (i,f,g,o) -> (i,f,o,g) along `axis` (size 4H)."""
    parts = np.split(a, 4, axis=axis)
    return np.concatenate([parts[0], parts[1], parts[3], parts[2]], axis=axis)


def _lhsT_tiles_mmajor(wT, kt):
    """wT: [K, 2048] -> [128, 16*kt*128] with free index (m, k, j)."""
    K = wT.shape[0]
    assert K == kt * 128
    a = wT.reshape(kt, 128, M_TILES, 128)       # [k, p, m, j]
    return np.ascontiguousarray(a.transpose(1, 2, 0, 3).reshape(128, kt * 2048))


def _build_program():
    import concourse.bass as bass
    import concourse.tile as tile
    import concourse.mybir as mybir
    from contextlib import ExitStack

    f16 = mybir.dt.float16
    f32 = mybir.dt.float32
    AF = mybir.ActivationFunctionType
    Alu = mybir.AluOpType

    class SplitDrainTileContext(tile.TileContext):
        """This container's walrus allows only one sync-wait per instruction;
        Tile's kernel-tail drain carries one wait per live semaphore.  Split
        it into a chain of single-wait drains (same semantics: by the last
        drain every semaphore has reached its target)."""

        def _drain_and_barrier(self, tick_clock, wait_clock):
            from concourse.vector_clock import ScopedClock
            drain_inst = self.nc.sync.drain()
            wait_clock.add_sem_waits(
                drain_inst.ins, ScopedClock({None: tick_clock.global_clock}))
            si = drain_inst.ins.sync_info
            waits = list(si.on_wait or []) if si is not None else []
            if len(waits) > 1:
                ups = list(si.on_update or [])
                drain_inst.ins.sync_info = mybir.SyncInfo(
                    on_wait=[waits[0]], on_update=ups)
                for w in waits[1:]:
                    d2 = self.nc.sync.drain()
                    d2.ins.sync_info = mybir.SyncInfo(on_wait=[w], on_update=[])
            self.nc.all_engine_barrier()
            popped = self.nc._tile_sem_poison_stack.pop()
            assert popped is self._sem_poison
            self.nc.clear_and_free_semaphores(list(self.sems.allocated().values()))
            self.nc.all_engine_barrier()

    nc = bass.Bass()

    w0T = nc.declare_dram_parameter("w0T", [128, 4 * 2048], f16, isOutput=False)
    w1iT = nc.declare_dram_parameter("w1iT", [128, 4 * 2048], f16, isOutput=False)
    w1hT = nc.declare_dram_parameter("w1hT", [128, 4 * 2048], f16, isOutput=False)
    wx4T = nc.declare_dram_parameter("wx4T", [4, 2048], f16, isOutput=False)
    wpT = nc.declare_dram_parameter("wpT", [128, 12], f16, isOutput=False)
    b1rd = nc.declare_dram_parameter("b1rep", [128, M_TILES * T], f16, isOutput=False)
    oh3d = nc.declare_dram_parameter("oh3", [3, 3 * 128], f16, isOutput=False)
    bpcd = nc.declare_dram_parameter("bpc84", [3, T], f32, isOutput=False)
    tvd = nc.declare_dram_parameter("tvals", [1, F_OUT], f16, isOutput=False)
    xsid = nc.declare_dram_parameter("xsinit", [4, T + 1], f16, isOutput=False)
    lensd = nc.declare_dram_parameter("lens", [NB], f32, isOutput=False)
    outd = nc.declare_dram_parameter("out", [NB, F_OUT], f32, isOutput=True)

    with ExitStack() as ctx:
        tc = ctx.enter_context(SplitDrainTileContext(nc))
        const = ctx.enter_context(tc.tile_pool(name="const", bufs=1))
        tmp = ctx.enter_context(tc.tile_pool(name="tmp", bufs=2))
        GA0 = ctx.enter_context(nc.psum_tensor([128, 1, 512], f32))
        GB0 = ctx.enter_context(nc.psum_tensor([128, 1, 512], f32))
        GO0 = ctx.enter_context(nc.psum_tensor([128, 1, 512], f32))
        GA1 = ctx.enter_context(nc.psum_tensor([128, 1, 512], f32))
        GB1 = ctx.enter_context(nc.psum_tensor([128, 1, 512], f32))
        GO1 = ctx.enter_context(nc.psum_tensor([128, 1, 512], f32))
        PB = ctx.enter_context(nc.psum_tensor([128, 1, 512], f32))
        BC = ctx.enter_context(nc.psum_tensor([128, 1, 512], f32))
        G0 = (GA0, GB0, GO0)
        G1 = (GA1, GB1, GO1)
        Pap = PB[0:3, 0, 0:T]              # head output

        _r2slot = {}
        for s, r in enumerate(SLOTS_A):
            _r2slot[r] = (0, s)
        for s, r in enumerate(SLOTS_B):
            _r2slot[r] = (1, s)
        for s, r in enumerate(SLOTS_O):
            _r2slot[r] = (2, s)

        def greg(G, r):
            ti, s = _r2slot[r]
            return G[ti][:, 0, s * T:(s + 1) * T]

        # ---- constants / weights into SBUF ----
        # weight DMAs are chunked and ordered by first use; each chunk's
        # semaphore is absorbed into the PE clock by a tiny ldweights right
        # before its first consuming matmul.
        wx4s = const.tile([4, 2048], f16)
        nc.sync.dma_start(wx4s[:], wx4T[:, :])
        w1is = const.tile([128, 4 * 2048], f16)
        for lo, hi in WCHUNKS:
            nc.sync.dma_start(w1is[:, lo * 512:hi * 512], w1iT[:, lo * 512:hi * 512])
        w0s = const.tile([128, 4 * 2048], f16)
        for lo, hi in WCHUNKS:
            nc.sync.dma_start(w0s[:, lo * 512:hi * 512], w0T[:, lo * 512:hi * 512])
        w1hs = const.tile([128, 4 * 2048], f16)
        for lo, hi in WCHUNKS:
            nc.sync.dma_start(w1hs[:, lo * 512:hi * 512], w1hT[:, lo * 512:hi * 512])
        wpss = const.tile([128, 12], f16)
        nc.sync.dma_start(wpss[:], wpT[:, :])
        b1rs = const.tile([128, M_TILES * T], f16)
        nc.sync.dma_start(b1rs[:], b1rd[:, :])
        oh3s = const.tile([3, 3 * 128], f16)
        nc.sync.dma_start(oh3s[:], oh3d[:, :])
        bpcs = const.tile([3, T], f32)
        nc.sync.dma_start(bpcs[:], bpcd[:, :])
        tvs = const.tile([1, F_OUT], f16)
        nc.sync.dma_start(tvs[:], tvd[:, :])
        Xs = const.tile([4, T + 1], f16)
        nc.sync.dma_start(Xs[:], xsid[:, :])  # rows 0:3 zero, row 3 = 1 (bias)
        lenss = const.tile([128, BT], f32)
        nc.sync.dma_start(lenss[:], lensd.rearrange("(m p) -> p m", p=128))

        ones1 = const.tile([1, 128], f16)
        nc.vector.memset(ones1[:], 1.0)

        # states: ping-pong buffers, col 0 = zero state
        H0s = [const.tile([128, 4, T + 1], f16, name=f"h0_{i}") for i in range(2)]
        H1s = [const.tile([128, 4, T + 1], f16, name=f"h1_{i}") for i in range(2)]
        for s in (*H0s, *H1s):
            nc.vector.memset(s[:, :, 0:1], 0.0)

        # DVE absorbers for DVE-consumed const DMAs
        absb = const.tile([1, 5], f32)
        nc.vector.tensor_copy(absb[:, 0:1], b1rs[0:1, 0:1])
        nc.vector.tensor_copy(absb[:, 1:2], bpcs[0:1, 0:1])
        nc.vector.tensor_copy(absb[:, 2:3], tvs[0:1, 0:1])
        nc.vector.tensor_copy(absb[:, 3:4], lenss[0:1, 0:1])
        nc.vector.tensor_copy(absb[:, 4:5], Xs[0:1, 0:1])
        nc.tensor.ldweights(wx4s[0:4, 0:1])

        # broadcast the t-values row across partitions (once)
        tvbc = const.tile([128, F_OUT], f32)
        nc.tensor.matmul(BC[:, 0, F_OUT:2 * F_OUT], lhsT=ones1[:], rhs=tvs[:],
                         start=True, stop=True)
        nc.vector.tensor_copy(tvbc[:], BC[:, 0, F_OUT:2 * F_OUT])

        def cell_elementwise_a(G, cellno, j, bias):
            """Part 1: after tensors A and B are complete -- tanh(g),
            sig(f,i), u, the c scan and tanh(c).  Runs while the o-gate
            matmuls stream through the PE."""
            GA, GB, _ = G
            gba = tmp.tile([128, 6 * T], f16, tag=f"gba{cellno}", bufs=2)
            if bias is not None:
                nc.vector.tensor_add(gba[:], GA[:, 0, 0:6 * T], bias[:, 0:6 * T])
            else:
                nc.vector.tensor_copy(gba[:], GA[:, 0, 0:6 * T])
            tg = tmp.tile([128, 4 * T], f16, tag=f"tg{cellno}_{j}", bufs=1)
            nc.scalar.activation(tg[:], gba[:, 0:4 * T], AF.Tanh)     # tanh(g)
            sgf01 = tmp.tile([128, 2 * T], f16, tag=f"sgf01_{cellno}_{j}", bufs=1)
            nc.scalar.activation(sgf01[:], gba[:, 4 * T:6 * T], AF.Sigmoid)
            gbb = tmp.tile([128, 6 * T], f16, tag=f"gbb{cellno}", bufs=2)
            if bias is not None:
                nc.vector.tensor_add(gbb[:], GB[:, 0, 0:6 * T], bias[:, 6 * T:12 * T])
            else:
                nc.vector.tensor_copy(gbb[:], GB[:, 0, 0:6 * T])
            sgb = tmp.tile([128, 6 * T], f16, tag=f"sgb{cellno}_{j}", bufs=1)
            nc.scalar.activation(sgb[:], gbb[:], AF.Sigmoid)    # sig(f23, i)
            u = tmp.tile([128, 4 * T], f16, tag=f"u{cellno}", bufs=2)
            nc.vector.tensor_mul(u[:], sgb[:, 2 * T:6 * T], tg[:])  # sig(i)*tanh(g)
            cf = tmp.tile([128, 4 * T], f16, tag=f"c{cellno}", bufs=2)
            for c4, sf in ((0, sgf01[:, 0:T]), (1, sgf01[:, T:2 * T]),
                           (2, sgb[:, 0:T]), (3, sgb[:, T:2 * T])):
                nc.vector.tensor_tensor_scan(
                    cf[:, c4 * T:(c4 + 1) * T], sf, u[:, c4 * T:(c4 + 1) * T],
                    0.0, Alu.mult, Alu.add)   # c_t = sig(f_t)*c_{t-1} + u_t
            tcn = tmp.tile([128, 4 * T], f16, tag=f"tc{cellno}_{j}", bufs=1)
            nc.scalar.activation(tcn[:], cf[:], AF.Tanh)
            return tcn

        def cell_elementwise_o(G, tcn, h_out, cellno, j, bias):
            """Part 2: after tensor O -- sig(o), h' = sig(o)*tanh(c)."""
            _, _, GO = G
            gbo = tmp.tile([128, 4 * T], f16, tag=f"gbo{cellno}", bufs=2)
            if bias is not None:
                nc.vector.tensor_add(gbo[:], GO[:, 0, 0:4 * T], bias[:, 12 * T:])
            else:
                nc.vector.tensor_copy(gbo[:], GO[:, 0, 0:4 * T])
            sgo = tmp.tile([128, 4 * T], f16, tag=f"sgo{cellno}_{j}", bufs=1)
            nc.scalar.activation(sgo[:], gbo[:], AF.Sigmoid)
            nc.vector.tensor_mul(h_out, sgo[:], tcn[:])

        def emit_head(h1buf):
            """P(:, t) = W_pc @ h1(t), then Xs[:, 1:] = P + b_pc (fp16)."""
            for k in range(4):
                nc.tensor.matmul(Pap, lhsT=wpss[:, 3 * k:3 * k + 3],
                                 rhs=h1buf[:, k, 1:T + 1],
                                 start=(k == 0), stop=(k == 3))
            nc.vector.tensor_add(Xs[0:3, 1:T + 1], Pap, bpcs[:])

        def absorb(ws, m):
            # absorb the DMA chunk whose first m-tile is m into the PE clock
            nc.tensor.ldweights(ws[:, m * 512:m * 512 + 1])

        for j in range(J_SWEEPS):
            r, w = j % 2, (j + 1) % 2
            if j > 0:
                if j == 1:
                    absorb(wpss, 0)
                emit_head(H1s[r])           # head of sweep j-1 -> x for sweep j
            # G0 = W_hh0 @ h0(prev, shifted) + W_ih0 @ x + b0 (K=4 x+bias
            # pass); one tight accumulation group per gate region.
            for si, m in enumerate(SLOTS):
                if j == 1 and si in (0, 4, 8, 12):
                    absorb(w0s, (m // 4) * 4)
                for k in range(4):
                    if j > 0:
                        nc.tensor.matmul(
                            greg(G0, m),
                            lhsT=w0s[:, m * 512 + k * 128:m * 512 + (k + 1) * 128],
                            rhs=H0s[r][:, k, 0:T],
                            start=(k == 0), stop=False)
                nc.tensor.matmul(
                    greg(G0, m),
                    lhsT=wx4s[0:4, m * 128:(m + 1) * 128],
                    rhs=Xs[0:4, 0:T],
                    start=(j == 0), stop=True)
                if si == len(SLOTS_A) + len(SLOTS_B) - 1:
                    tcn0 = cell_elementwise_a(G0, 0, j, None)
            cell_elementwise_o(G0, tcn0, H0s[w][:, :, 1:T + 1], 0, j, None)
            # G1 = W_ih1 @ h0(this sweep) + W_hh1 @ h1(prev, shifted)
            for si, m in enumerate(SLOTS):
                if j == 0 and si in (0, 4, 8, 12):
                    absorb(w1is, (m // 4) * 4)
                if j == 1 and si in (0, 4, 8, 12):
                    absorb(w1hs, (m // 4) * 4)
                for k in range(4):
                    nc.tensor.matmul(
                        greg(G1, m),
                        lhsT=w1is[:, m * 512 + k * 128:m * 512 + (k + 1) * 128],
                        rhs=H0s[w][:, k, 1:T + 1],
                        start=(k == 0), stop=(j == 0 and k == 3))
                if j > 0:
                    for k in range(4):
                        nc.tensor.matmul(
                            greg(G1, m),
                            lhsT=w1hs[:, m * 512 + k * 128:m * 512 + (k + 1) * 128],
                            rhs=H1s[r][:, k, 0:T],
                            start=False, stop=(k == 3))
                if si == len(SLOTS_A) + len(SLOTS_B) - 1:
                    tcn1 = cell_elementwise_a(G1, 1, j, b1rs)
            cell_elementwise_o(G1, tcn1, H1s[w][:, :, 1:T + 1], 1, j, b1rs)

        # final head -> Xs[0:3, 1:85] = final trajectory points (with bias)
        emit_head(H1s[J_SWEEPS % 2])

        # ---- broadcast + mask + store ----
        nc.tensor.ldweights(oh3s[0:3, 0:1])   # absorb oh3 DMA sem into PE
        for i in range(3):
            nc.tensor.matmul(BC[:, 0, i * T:(i + 1) * T],
                             lhsT=oh3s[0:3, i * 128:(i + 1) * 128],
                             rhs=Xs[0:3, 1:T + 1], start=True, stop=True)
        pq = const.tile([128, T, 3], f32)     # interleaved [t, i] -> col 3t+i
        for i in range(3):
            nc.vector.tensor_copy(pq[:, :, i], BC[:, 0, i * T:(i + 1) * T])
        ot = const.tile([128, BT * F_OUT], f32)
        out_r = outd.rearrange("(n p) f -> p n f", p=128)
        for n in range(BT):
            # out_row = (tvals < len) * p_broadcast, fused in one DVE op
            nc.vector.scalar_tensor_tensor(
                ot[:, n * F_OUT:(n + 1) * F_OUT], tvbc[:], lenss[:, n:n + 1],
                pq[:, :, :], Alu.is_lt, Alu.mult)
            # chunked stores overlap the remaining masking ops; more/finer
            # DMAs would need ring-flow waits on top of the data wait, and
            # walrus allows only one sync wait per instruction
            if n % 4 == 3:
                nc.gpsimd.dma_start(
                    out_r[:, n - 3:n + 1, :],
                    ot[:, (n - 3) * F_OUT:(n + 1) * F_OUT])

    return nc


def _prep_inputs(inputs):
    f = lambda k: np.asarray(inputs[k], np.float32)
    Wih0 = _gate_reorder(f("W_ih0"))
    Whh0 = _gate_reorder(f("W_hh0"))
    Wih1 = _gate_reorder(f("W_ih1"))
    Whh1 = _gate_reorder(f("W_hh1"))
    b0 = _gate_reorder(f("b_ih0") + f("b_hh0"))
    b1 = _gate_reorder(f("b_ih1") + f("b_hh1"))
    Wpc = f("W_pc")
    bpc = f("b_pc")

    oh3 = np.zeros((3, 3 * 128), np.float16)
    for i in range(3):
        oh3[i, i * 128:(i + 1) * 128] = 1.0

    # b1 replicated across T in SLOT order: col s*T+t = b1[SLOTS[s]*128+p]
    b1m = b1.reshape(M_TILES, 128)[list(SLOTS)]           # [16 slots, 128]
    b1rep = np.repeat(b1m.T[:, :, None], T, axis=2).reshape(128, M_TILES * T)

    common = {
        "w0T": _lhsT_tiles_mmajor(Whh0.T.copy(), 4).astype(np.float16),
        "w1iT": _lhsT_tiles_mmajor(Wih1.T.copy(), 4).astype(np.float16),
        "w1hT": _lhsT_tiles_mmajor(Whh1.T.copy(), 4).astype(np.float16),
        "wx4T": np.ascontiguousarray(
            np.concatenate([Wih0.T, b0[None, :]], 0)).astype(np.float16),
        "wpT": np.ascontiguousarray(
            Wpc.T.reshape(4, 128, 3).transpose(1, 0, 2).reshape(128, 12)
        ).astype(np.float16),
        "b1rep": np.ascontiguousarray(b1rep).astype(np.float16),
        "oh3": oh3,
        "bpc84": np.ascontiguousarray(np.repeat(bpc[:, None], T, axis=1)),
        "tvals": np.repeat(np.arange(T, dtype=np.float16), IN).reshape(1, F_OUT),
        "xsinit": np.concatenate(
            [np.zeros((3, T + 1), np.float16), np.ones((1, T + 1), np.float16)], 0),
    }
    lens = np.asarray(inputs["seq_lengths"]).astype(np.float32)
    in_maps = []
    for c in range(N_CORES):
        m = dict(common)
        m["lens"] = np.ascontiguousarray(lens[c * NB:(c + 1) * NB])
        in_maps.append(m)
    return in_maps


def kernel(**inputs):
    global _COMPILED, LAST_RESULTS
    from concourse.bass_utils import run_bass_kernel_spmd

    if _COMPILED is None:
        _COMPILED = _build_program()
    nc = _COMPILED

    in_maps = _prep_inputs(inputs)
    res = run_bass_kernel_spmd(nc, in_maps, list(range(N_CORES)))
    LAST_RESULTS = res
    out = np.concatenate([res.results[c]["out"] for c in range(N_CORES)], axis=0)
    return np.ascontiguousarray(out.reshape(B, T, IN))


# revision 30
# speedup vs baseline: 4.9970x; 1.0104x over previous
"""Trainium2 Bass kernel for nn_DecoderLSTM.

Key observation: the reference module never reads `features` -- the LSTM input
starts at zeros and is fed back from the predicted point, and h/c start at
zeros.  Every batch row therefore computes the *identical* trajectory
p[t] (t=0..83); the per-row output is just p[t] masked by t < seq_lengths[b].

The sequential 84-step scan is replaced by a parallel-in-time Gauss-Seidel
iteration: all 84 timesteps are updated simultaneously (matmuls with N=84
moving columns), and the linear cell-state recurrence
c_t = sig(f_t)*c_{t-1} + sig(i_t)*tanh(g_t) is solved exactly within each
sweep by the DVE's tensor_tensor_scan.  Convergence (verified against the
host oracle): rel err 2.3e-3 after 4 sweeps, 8.8e-4 after 5, 2.0e-4 after 6.
The sequential version pays the PE weight-load for every 128x128 tile at
every one of 84 steps (~40ns/tile, 216 tiles/step -> ~700us); here each
weight tile is loaded once per sweep and serves all 84 columns.

Hardware rules this code is shaped around:
  - walrus allows ONE sync-wait per instruction.  Tile chains PSUM readers
    with sync edges at tensor granularity, so every PSUM tensor has exactly
    one reader instruction per sweep (a DVE copy/bias-add); ACT never reads
    PSUM and every ACT-written tile is fresh per sweep (ACT-ACT WAW edges
    also emit waits).
  - only one PSUM accumulation group may be open per 2KB bank, so each gate
    region's matmuls are emitted as one tight group.
  - gates are grouped into PSUM tensors by gate type, ordered (g,f01 |
    f23,i | o) so the o-gate matmuls stream while the scan/tanh chain runs:
    the PE's post-cell stall is only gb_o -> sig_o -> h'.

Layouts (per core):
  - states H0s/H1s (ping-pong pair): [128, 4, 85] fp16; col tau holds
    h(tau-1), col 0 is the t=-1 zero state; chunk k on dim1: h[128k+p].
  - x feed Xs: [4, 85] fp16; rows 0:3 = x (col tau = p_{tau-1}), row 3 = 1
    (carries the cell-0 bias through the x matmul, K=4).
  - gate region r = q*4 + ml (gate q of (i,f,o,g), h-chunk ml), 84 columns
    each, distributed across three PSUM tensors per cell in SLOT order
    (12,13,14,15,4,5 | 6,7,0,1,2,3 | 8,9,10,11).
  - weights: lhsT tiles [K=128, M=128] fp16, m-major (free = m*512+k*128+j),
    DMA'd in 4 chunks ordered by first use.
"""

import numpy as np

B = 16384
H = 512
T = 84
IN = 3
N_CORES = 8
NB = B // N_CORES          # 2048 rows per core
M_TILES = 16               # 2048 gate dims / 128
BT = NB // 128             # 16 batch tiles per core
F_OUT = T * IN             # 252
J_SWEEPS = 5

# gate regions in emission (slot) order: tensor A = g gates + f chunks 0,1;
# tensor B = f chunks 2,3 + i gates; tensor O = o gates (streamed last)
SLOTS_A = (12, 13, 14, 15, 4, 5)
SLOTS_B = (6, 7, 0, 1, 2, 3)
SLOTS_O = (8, 9, 10, 11)
SLOTS = SLOTS_A + SLOTS_B + SLOTS_O
# weight DMA chunks (4 m-tiles each) in first-use order of SLOTS
WCHUNKS = ((12, 16), (4, 8), (0, 4), (8, 12))

_COMPILED = None           # nc cache
LAST_RESULTS = None        # BassKernelResults from the last run (for test.py)


def _gate_reorder(a, axis=0):
    """torch gate order (i,f,g,o) -> (i,f,o,g) along `axis` (size 4H)."""
    parts = np.split(a, 4, axis=axis)
    return np.concatenate([parts[0], parts[1], parts[3], parts[2]], axis=axis)


def _lhsT_tiles_mmajor(wT, kt):
    """wT: [K, 2048] -> [128, 16*kt*128] with free index (m, k, j)."""
    K = wT.shape[0]
    assert K == kt * 128
    a = wT.reshape(kt, 128, M_TILES, 128)       # [k, p, m, j]
    return np.ascontiguousarray(a.transpose(1, 2, 0, 3).reshape(128, kt * 2048))


def _build_program():
    import concourse.bass as bass
    import concourse.tile as tile
    import concourse.mybir as mybir
    from contextlib import ExitStack

    f16 = mybir.dt.float16
    f32 = mybir.dt.float32
    AF = mybir.ActivationFunctionType
    Alu = mybir.AluOpType

    class SplitDrainTileContext(tile.TileContext):
        """This container's walrus allows only one sync-wait per instruction;
        Tile's kernel-tail drain carries one wait per live semaphore.  Split
        it into a chain of single-wait drains (same semantics: by the last
        drain every semaphore has reached its target)."""

        def _drain_and_barrier(self, tick_clock, wait_clock):
            from concourse.vector_clock import ScopedClock
            drain_inst = self.nc.sync.drain()
            wait_clock.add_sem_waits(
                drain_inst.ins, ScopedClock({None: tick_clock.global_clock}))
            si = drain_inst.ins.sync_info
            waits = list(si.on_wait or []) if si is not None else []
            if len(waits) > 1:
                ups = list(si.on_update or [])
                drain_inst.ins.sync_info = mybir.SyncInfo(
                    on_wait=[waits[0]], on_update=ups)
                for w in waits[1:]:
                    d2 = self.nc.sync.drain()
                    d2.ins.sync_info = mybir.SyncInfo(on_wait=[w], on_update=[])
            self.nc.all_engine_barrier()
            popped = self.nc._tile_sem_poison_stack.pop()
            assert popped is self._sem_poison
            self.nc.clear_and_free_semaphores(list(self.sems.allocated().values()))
            self.nc.all_engine_barrier()

    nc = bass.Bass()

    w0T = nc.declare_dram_parameter("w0T", [128, 4 * 2048], f16, isOutput=False)
    w1iT = nc.declare_dram_parameter("w1iT", [128, 4 * 2048], f16, isOutput=False)
    w1hT = nc.declare_dram_parameter("w1hT", [128, 4 * 2048], f16, isOutput=False)
    wx4T = nc.declare_dram_parameter("wx4T", [4, 2048], f16, isOutput=False)
    wpT = nc.declare_dram_parameter("wpT", [128, 12], f16, isOutput=False)
    b1rd = nc.declare_dram_parameter("b1rep", [128, M_TILES * T], f16, isOutput=False)
    oh3d = nc.declare_dram_parameter("oh3", [3, 3 * 128], f16, isOutput=False)
    bpcd = nc.declare_dram_parameter("bpc84", [3, T], f32, isOutput=False)
    tvd = nc.declare_dram_parameter("tvals", [1, F_OUT], f16, isOutput=False)
    xsid = nc.declare_dram_parameter("xsinit", [4, T + 1], f16, isOutput=False)
    lensd = nc.declare_dram_parameter("lens", [NB], f32, isOutput=False)
    outd = nc.declare_dram_parameter("out", [NB, F_OUT], f32, isOutput=True)

    with ExitStack() as ctx:
        tc = ctx.enter_context(SplitDrainTileContext(nc))
        const = ctx.enter_context(tc.tile_pool(name="const", bufs=1))
        tmp = ctx.enter_context(tc.tile_pool(name="tmp", bufs=2))
        GA0 = ctx.enter_context(nc.psum_tensor([128, 1, 512], f32))
        GB0 = ctx.enter_context(nc.psum_tensor([128, 1, 512], f32))
        GO0 = ctx.enter_context(nc.psum_tensor([128, 1, 512], f32))
        GA1 = ctx.enter_context(nc.psum_tensor([128, 1, 512], f32))
        GB1 = ctx.enter_context(nc.psum_tensor([128, 1, 512], f32))
        GO1 = ctx.enter_context(nc.psum_tensor([128, 1, 512], f32))
        PB = ctx.enter_context(nc.psum_tensor([128, 1, 512], f32))
        BC = ctx.enter_context(nc.psum_tensor([128, 1, 512], f32))
        G0 = (GA0, GB0, GO0)
        G1 = (GA1, GB1, GO1)
        Pap = PB[0:3, 0, 0:T]              # head output

        _r2slot = {}
        for s, r in enumerate(SLOTS_A):
            _r2slot[r] = (0, s)
        for s, r in enumerate(SLOTS_B):
            _r2slot[r] = (1, s)
        for s, r in enumerate(SLOTS_O):
            _r2slot[r] = (2, s)

        def greg(G, r):
            ti, s = _r2slot[r]
            return G[ti][:, 0, s * T:(s + 1) * T]

        # ---- constants / weights into SBUF ----
        # weight DMAs are chunked and ordered by first use; each chunk's
        # semaphore is absorbed into the PE clock by a tiny ldweights right
        # before its first consuming matmul.
        # DMA ring-flow waits are cumulative per queue and walrus allows
        # only one sync wait per instruction, so the gpsimd queue is
        # reserved for the four output stores; loads alternate between the
        # SP and ACT queues so weight chunks land in parallel and sweeps
        # 0/1 are not DMA-starved (6.4MB of weights vs ~200GB/s per queue).
        queues = (nc.sync.dma_start, nc.scalar.dma_start)
        wx4s = const.tile([4, 2048], f16)
        queues[0](wx4s[:], wx4T[:, :])
        Xs = const.tile([4, T + 1], f16)
        queues[1](Xs[:], xsid[:, :])  # rows 0:3 zero, row 3 = 1 (bias)
        b1rs = const.tile([128, M_TILES * T], f16)
        queues[0](b1rs[:], b1rd[:, :])
        wpss = const.tile([128, 12], f16)
        queues[0](wpss[:], wpT[:, :])
        w1is = const.tile([128, 4 * 2048], f16)
        for qi, (lo, hi) in enumerate(WCHUNKS):
            queues[qi % 2](w1is[:, lo * 512:hi * 512], w1iT[:, lo * 512:hi * 512])
        w0s = const.tile([128, 4 * 2048], f16)
        for qi, (lo, hi) in enumerate(WCHUNKS):
            queues[(qi + 1) % 2](w0s[:, lo * 512:hi * 512], w0T[:, lo * 512:hi * 512])
        w1hs = const.tile([128, 4 * 2048], f16)
        for qi, (lo, hi) in enumerate(WCHUNKS):
            queues[qi % 2](w1hs[:, lo * 512:hi * 512], w1hT[:, lo * 512:hi * 512])
        oh3s = const.tile([3, 3 * 128], f16)
        queues[1](oh3s[:], oh3d[:, :])
        bpcs = const.tile([3, T], f32)
        queues[1](bpcs[:], bpcd[:, :])
        tvs = const.tile([1, F_OUT], f16)
        queues[0](tvs[:], tvd[:, :])
        lenss = const.tile([128, BT], f32)
        queues[1](lenss[:], lensd.rearrange("(m p) -> p m", p=128))

        ones1 = const.tile([1, 128], f16)
        nc.vector.memset(ones1[:], 1.0)

        # states: ping-pong buffers, col 0 = zero state
        H0s = [const.tile([128, 4, T + 1], f16, name=f"h0_{i}") for i in range(2)]
        H1s = [const.tile([128, 4, T + 1], f16, name=f"h1_{i}") for i in range(2)]
        for s in (*H0s, *H1s):
            nc.vector.memset(s[:, :, 0:1], 0.0)

        # DVE absorbers for DVE-consumed const DMAs
        absb = const.tile([1, 5], f32)
        nc.vector.tensor_copy(absb[:, 0:1], b1rs[0:1, 0:1])
        nc.vector.tensor_copy(absb[:, 1:2], bpcs[0:1, 0:1])
        nc.vector.tensor_copy(absb[:, 2:3], tvs[0:1, 0:1])
        nc.vector.tensor_copy(absb[:, 3:4], lenss[0:1, 0:1])
        nc.vector.tensor_copy(absb[:, 4:5], Xs[0:1, 0:1])
        nc.tensor.ldweights(wx4s[0:4, 0:1])

        # broadcast the t-values row across partitions (once)
        tvbc = const.tile([128, F_OUT], f32)
        nc.tensor.matmul(BC[:, 0, F_OUT:2 * F_OUT], lhsT=ones1[:], rhs=tvs[:],
                         start=True, stop=True)
        nc.vector.tensor_copy(tvbc[:], BC[:, 0, F_OUT:2 * F_OUT])

        def cell_elementwise_a(G, cellno, j, bias):
            """Part 1: after tensors A and B are complete -- tanh(g),
            sig(f,i), u, the c scan and tanh(c).  Runs while the o-gate
            matmuls stream through the PE."""
            GA, GB, _ = G
            gba = tmp.tile([128, 6 * T], f16, tag=f"gba{cellno}", bufs=2)
            if bias is not None:
                nc.vector.tensor_add(gba[:], GA[:, 0, 0:6 * T], bias[:, 0:6 * T])
            else:
                nc.vector.tensor_copy(gba[:], GA[:, 0, 0:6 * T])
            tg = tmp.tile([128, 4 * T], f16, tag=f"tg{cellno}_{j}", bufs=1)
            nc.scalar.activation(tg[:], gba[:, 0:4 * T], AF.Tanh)     # tanh(g)
            sgf01 = tmp.tile([128, 2 * T], f16, tag=f"sgf01_{cellno}_{j}", bufs=1)
            nc.scalar.activation(sgf01[:], gba[:, 4 * T:6 * T], AF.Sigmoid)
            gbb = tmp.tile([128, 6 * T], f16, tag=f"gbb{cellno}", bufs=2)
            if bias is not None:
                nc.vector.tensor_add(gbb[:], GB[:, 0, 0:6 * T], bias[:, 6 * T:12 * T])
            else:
                nc.vector.tensor_copy(gbb[:], GB[:, 0, 0:6 * T])
            sgb = tmp.tile([128, 6 * T], f16, tag=f"sgb{cellno}_{j}", bufs=1)
            nc.scalar.activation(sgb[:], gbb[:], AF.Sigmoid)    # sig(f23, i)
            u = tmp.tile([128, 4 * T], f16, tag=f"u{cellno}", bufs=2)
            nc.vector.tensor_mul(u[:], sgb[:, 2 * T:6 * T], tg[:])  # sig(i)*tanh(g)
            cf = tmp.tile([128, 4 * T], f16, tag=f"c{cellno}", bufs=2)
            for c4, sf in ((0, sgf01[:, 0:T]), (1, sgf01[:, T:2 * T]),
                           (2, sgb[:, 0:T]), (3, sgb[:, T:2 * T])):
                nc.vector.tensor_tensor_scan(
                    cf[:, c4 * T:(c4 + 1) * T], sf, u[:, c4 * T:(c4 + 1) * T],
                    0.0, Alu.mult, Alu.add)   # c_t = sig(f_t)*c_{t-1} + u_t
            tcn = tmp.tile([128, 4 * T], f16, tag=f"tc{cellno}_{j}", bufs=1)
            nc.scalar.activation(tcn[:], cf[:], AF.Tanh)
            return tcn

        def cell_elementwise_o(G, tcn, h_out, cellno, j, bias):
            """Part 2: after tensor O -- sig(o), h' = sig(o)*tanh(c)."""
            _, _, GO = G
            gbo = tmp.tile([128, 4 * T], f16, tag=f"gbo{cellno}", bufs=2)
            if bias is not None:
                nc.vector.tensor_add(gbo[:], GO[:, 0, 0:4 * T], bias[:, 12 * T:])
            else:
                nc.vector.tensor_copy(gbo[:], GO[:, 0, 0:4 * T])
            sgo = tmp.tile([128, 4 * T], f16, tag=f"sgo{cellno}_{j}", bufs=1)
            nc.scalar.activation(sgo[:], gbo[:], AF.Sigmoid)
            nc.vector.tensor_mul(h_out, sgo[:], tcn[:])

        def emit_head(h1buf):
            """P(:, t) = W_pc @ h1(t), then Xs[:, 1:] = P + b_pc (fp16)."""
            for k in range(4):
                nc.tensor.matmul(Pap, lhsT=wpss[:, 3 * k:3 * k + 3],
                                 rhs=h1buf[:, k, 1:T + 1],
                                 start=(k == 0), stop=(k == 3))
            nc.vector.tensor_add(Xs[0:3, 1:T + 1], Pap, bpcs[:])

        def absorb(ws, m):
            # absorb the DMA chunk whose first m-tile is m into the PE clock
            nc.tensor.ldweights(ws[:, m * 512:m * 512 + 1])

        for j in range(J_SWEEPS):
            r, w = j % 2, (j + 1) % 2
            if j > 0:
                if j == 1:
                    absorb(wpss, 0)
                emit_head(H1s[r])           # head of sweep j-1 -> x for sweep j
            # G0 = W_hh0 @ h0(prev, shifted) + W_ih0 @ x + b0 (K=4 x+bias
            # pass); one tight accumulation group per gate region.
            for si, m in enumerate(SLOTS):
                if j == 1 and si in (0, 4, 8, 12):
                    absorb(w0s, (m // 4) * 4)
                for k in range(4):
                    if j > 0:
                        nc.tensor.matmul(
                            greg(G0, m),
                            lhsT=w0s[:, m * 512 + k * 128:m * 512 + (k + 1) * 128],
                            rhs=H0s[r][:, k, 0:T],
                            start=(k == 0), stop=False)
                nc.tensor.matmul(
                    greg(G0, m),
                    lhsT=wx4s[0:4, m * 128:(m + 1) * 128],
                    rhs=Xs[0:4, 0:T],
                    start=(j == 0), stop=True)
                if si == len(SLOTS_A) + len(SLOTS_B) - 1:
                    tcn0 = cell_elementwise_a(G0, 0, j, None)
            cell_elementwise_o(G0, tcn0, H0s[w][:, :, 1:T + 1], 0, j, None)
            # G1 = W_ih1 @ h0(this sweep) + W_hh1 @ h1(prev, shifted)
            for si, m in enumerate(SLOTS):
                if j == 0 and si in (0, 4, 8, 12):
                    absorb(w1is, (m // 4) * 4)
                if j == 1 and si in (0, 4, 8, 12):
                    absorb(w1hs, (m // 4) * 4)
                for k in range(4):
                    nc.tensor.matmul(
                        greg(G1, m),
                        lhsT=w1is[:, m * 512 + k * 128:m * 512 + (k + 1) * 128],
                        rhs=H0s[w][:, k, 1:T + 1],
                        start=(k == 0), stop=(j == 0 and k == 3))
                if j > 0:
                    for k in range(4):
                        nc.tensor.matmul(
                            greg(G1, m),
                            lhsT=w1hs[:, m * 512 + k * 128:m * 512 + (k + 1) * 128],
                            rhs=H1s[r][:, k, 0:T],
                            start=False, stop=(k == 3))
                if si == len(SLOTS_A) + len(SLOTS_B) - 1:
                    tcn1 = cell_elementwise_a(G1, 1, j, b1rs)
            cell_elementwise_o(G1, tcn1, H1s[w][:, :, 1:T + 1], 1, j, b1rs)

        # final head -> Xs[0:3, 1:85] = final trajectory points (with bias)
        emit_head(H1s[J_SWEEPS % 2])

        # ---- broadcast + mask + store ----
        nc.tensor.ldweights(oh3s[0:3, 0:1])   # absorb oh3 DMA sem into PE
        for i in range(3):
            nc.tensor.matmul(BC[:, 0, i * T:(i + 1) * T],
                             lhsT=oh3s[0:3, i * 128:(i + 1) * 128],
                             rhs=Xs[0:3, 1:T + 1], start=True, stop=True)
        pq = const.tile([128, T, 3], f32)     # interleaved [t, i] -> col 3t+i
        for i in range(3):
            nc.vector.tensor_copy(pq[:, :, i], BC[:, 0, i * T:(i + 1) * T])
        ot = const.tile([128, BT * F_OUT], f32)
        out_r = outd.rearrange("(n p) f -> p n f", p=128)
        for n in range(BT):
            # out_row = (tvals < len) * p_broadcast, fused in one DVE op
            nc.vector.scalar_tensor_tensor(
                ot[:, n * F_OUT:(n + 1) * F_OUT], tvbc[:], lenss[:, n:n + 1],
                pq[:, :, :], Alu.is_lt, Alu.mult)
            # chunked stores overlap the remaining masking ops, one chunk per
            # DMA queue (finer per-queue chunks would need ring-flow waits on
            # top of the data wait; walrus allows one sync wait each)
            if n % 4 == 3:
                nc.gpsimd.dma_start(
                    out_r[:, n - 3:n + 1, :],
                    ot[:, (n - 3) * F_OUT:(n + 1) * F_OUT])

    return nc


def _prep_inputs(inputs):
    f = lambda k: np.asarray(inputs[k], np.float32)
    Wih0 = _gate_reorder(f("W_ih0"))
    Whh0 = _gate_reorder(f("W_hh0"))
    Wih1 = _gate_reorder(f("W_ih1"))
    Whh1 = _gate_reorder(f("W_hh1"))
    b0 = _gate_reorder(f("b_ih0") + f("b_hh0"))
    b1 = _gate_reorder(f("b_ih1") + f("b_hh1"))
    Wpc = f("W_pc")
    bpc = f("b_pc")

    oh3 = np.zeros((3, 3 * 128), np.float16)
    for i in range(3):
        oh3[i, i * 128:(i + 1) * 128] = 1.0

    # b1 replicated across T in SLOT order: col s*T+t = b1[SLOTS[s]*128+p]
    b1m = b1.reshape(M_TILES, 128)[list(SLOTS)]           # [16 slots, 128]
    b1rep = np.repeat(b1m.T[:, :, None], T, axis=2).reshape(128, M_TILES * T)

    common = {
        "w0T": _lhsT_tiles_mmajor(Whh0.T.copy(), 4).astype(np.float16),
        "w1iT": _lhsT_tiles_mmajor(Wih1.T.copy(), 4).astype(np.float16),
        "w1hT": _lhsT_tiles_mmajor(Whh1.T.copy(), 4).astype(np.float16),
        "wx4T": np.ascontiguousarray(
            np.concatenate([Wih0.T, b0[None, :]], 0)).astype(np.float16),
        "wpT": np.ascontiguousarray(
            Wpc.T.reshape(4, 128, 3).transpose(1, 0, 2).reshape(128, 12)
        ).astype(np.float16),
        "b1rep": np.ascontiguousarray(b1rep).astype(np.float16),
        "oh3": oh3,
        "bpc84": np.ascontiguousarray(np.repeat(bpc[:, None], T, axis=1)),
        "tvals": np.repeat(np.arange(T, dtype=np.float16), IN).reshape(1, F_OUT),
        "xsinit": np.concatenate(
            [np.zeros((3, T + 1), np.float16), np.ones((1, T + 1), np.float16)], 0),
    }
    lens = np.asarray(inputs["seq_lengths"]).astype(np.float32)
    in_maps = []
    for c in range(N_CORES):
        m = dict(common)
        m["lens"] = np.ascontiguousarray(lens[c * NB:(c + 1) * NB])
        in_maps.append(m)
    return in_maps


def kernel(**inputs):
    global _COMPILED, LAST_RESULTS
    from concourse.bass_utils import run_bass_kernel_spmd

    if _COMPILED is None:
        _COMPILED = _build_program()
    nc = _COMPILED

    in_maps = _prep_inputs(inputs)
    res = run_bass_kernel_spmd(nc, in_maps, list(range(N_CORES)))
    LAST_RESULTS = res
    out = np.concatenate([res.results[c]["out"] for c in range(N_CORES)], axis=0)
    return np.ascontiguousarray(out.reshape(B, T, IN))


# revision 31
# speedup vs baseline: 5.7858x; 1.1579x over previous
"""Trainium2 Bass kernel for nn_DecoderLSTM.

Key observation: the reference module never reads `features` -- the LSTM input
starts at zeros and is fed back from the predicted point, and h/c start at
zeros.  Every batch row therefore computes the *identical* trajectory
p[t] (t=0..83); the per-row output is just p[t] masked by t < seq_lengths[b].

The sequential 84-step scan is replaced by a parallel-in-time Gauss-Seidel
iteration: all 84 timesteps are updated simultaneously (matmuls with N=84
moving columns), and the linear cell-state recurrence
c_t = sig(f_t)*c_{t-1} + sig(i_t)*tanh(g_t) is solved exactly within each
sweep by the DVE's tensor_tensor_scan.  Convergence (verified against the
host oracle): rel err 2.3e-3 after 4 sweeps, 8.8e-4 after 5, 2.0e-4 after 6.
The sequential version pays the PE weight-load for every 128x128 tile at
every one of 84 steps (~40ns/tile, 216 tiles/step -> ~700us); here each
weight tile is loaded once per sweep and serves all 84 columns.

Hardware rules this code is shaped around:
  - walrus allows ONE sync-wait per instruction.  Tile chains PSUM readers
    with sync edges at tensor granularity, so every PSUM tensor has exactly
    one reader instruction per sweep (a DVE copy/bias-add); ACT never reads
    PSUM and every ACT-written tile is fresh per sweep (ACT-ACT WAW edges
    also emit waits).
  - only one PSUM accumulation group may be open per 2KB bank, so each gate
    region's matmuls are emitted as one tight group.
  - gates are grouped into PSUM tensors by gate type, ordered (g,f01 |
    f23,i | o) so the o-gate matmuls stream while the scan/tanh chain runs:
    the PE's post-cell stall is only gb_o -> sig_o -> h'.

Layouts (per core):
  - states H0s/H1s (ping-pong pair): [128, 4, 85] fp16; col tau holds
    h(tau-1), col 0 is the t=-1 zero state; chunk k on dim1: h[128k+p].
  - x feed Xs: [4, 85] fp16; rows 0:3 = x (col tau = p_{tau-1}), row 3 = 1
    (carries the cell-0 bias through the x matmul, K=4).
  - gate region r = q*4 + ml (gate q of (i,f,o,g), h-chunk ml), 84 columns
    each, distributed across three PSUM tensors per cell in SLOT order
    (12,13,14,15,4,5 | 6,7,0,1,2,3 | 8,9,10,11).
  - weights: lhsT tiles [K=128, M=128] fp16, m-major (free = m*512+k*128+j),
    DMA'd in 4 chunks ordered by first use.
"""

import numpy as np

B = 16384
H = 512
T = 84
IN = 3
N_CORES = 8
NB = B // N_CORES          # 2048 rows per core
M_TILES = 16               # 2048 gate dims / 128
BT = NB // 128             # 16 batch tiles per core
F_OUT = T * IN             # 252
J_SWEEPS = 4

# gate regions in emission (slot) order: tensor A = g gates + f chunks 0,1;
# tensor B = f chunks 2,3 + i gates; tensor O = o gates (streamed last)
SLOTS_A = (12, 13, 14, 15, 4, 5)
SLOTS_B = (6, 7, 0, 1, 2, 3)
SLOTS_O = (8, 9, 10, 11)
SLOTS = SLOTS_A + SLOTS_B + SLOTS_O
# weight DMA chunks (4 m-tiles each) in first-use order of SLOTS
WCHUNKS = ((12, 16), (4, 8), (0, 4), (8, 12))

_COMPILED = None           # nc cache
LAST_RESULTS = None        # BassKernelResults from the last run (for test.py)


def _gate_reorder(a, axis=0):
    """torch gate order (i,f,g,o) -> (i,f,o,g) along `axis` (size 4H)."""
    parts = np.split(a, 4, axis=axis)
    return np.concatenate([parts[0], parts[1], parts[3], parts[2]], axis=axis)


def _lhsT_tiles_mmajor(wT, kt):
    """wT: [K, 2048] -> [128, 16*kt*128] with free index (m, k, j)."""
    K = wT.shape[0]
    assert K == kt * 128
    a = wT.reshape(kt, 128, M_TILES, 128)       # [k, p, m, j]
    return np.ascontiguousarray(a.transpose(1, 2, 0, 3).reshape(128, kt * 2048))


def _build_program():
    import concourse.bass as bass
    import concourse.tile as tile
    import concourse.mybir as mybir
    from contextlib import ExitStack

    f16 = mybir.dt.float16
    f32 = mybir.dt.float32
    AF = mybir.ActivationFunctionType
    Alu = mybir.AluOpType

    class SplitDrainTileContext(tile.TileContext):
        """This container's walrus allows only one sync-wait per instruction;
        Tile's kernel-tail drain carries one wait per live semaphore.  Split
        it into a chain of single-wait drains (same semantics: by the last
        drain every semaphore has reached its target)."""

        def _drain_and_barrier(self, tick_clock, wait_clock):
            from concourse.vector_clock import ScopedClock
            drain_inst = self.nc.sync.drain()
            wait_clock.add_sem_waits(
                drain_inst.ins, ScopedClock({None: tick_clock.global_clock}))
            si = drain_inst.ins.sync_info
            waits = list(si.on_wait or []) if si is not None else []
            if len(waits) > 1:
                ups = list(si.on_update or [])
                drain_inst.ins.sync_info = mybir.SyncInfo(
                    on_wait=[waits[0]], on_update=ups)
                for w in waits[1:]:
                    d2 = self.nc.sync.drain()
                    d2.ins.sync_info = mybir.SyncInfo(on_wait=[w], on_update=[])
            self.nc.all_engine_barrier()
            popped = self.nc._tile_sem_poison_stack.pop()
            assert popped is self._sem_poison
            self.nc.clear_and_free_semaphores(list(self.sems.allocated().values()))
            self.nc.all_engine_barrier()

    nc = bass.Bass()

    w0T = nc.declare_dram_parameter("w0T", [128, 4 * 2048], f16, isOutput=False)
    w1iT = nc.declare_dram_parameter("w1iT", [128, 4 * 2048], f16, isOutput=False)
    w1hT = nc.declare_dram_parameter("w1hT", [128, 4 * 2048], f16, isOutput=False)
    wx4T = nc.declare_dram_parameter("wx4T", [4, 2048], f16, isOutput=False)
    wpT = nc.declare_dram_parameter("wpT", [128, 12], f16, isOutput=False)
    b1rd = nc.declare_dram_parameter("b1rep", [128, M_TILES * T], f16, isOutput=False)
    oh3d = nc.declare_dram_parameter("oh3", [3, 3 * 128], f16, isOutput=False)
    bpcd = nc.declare_dram_parameter("bpc84", [3, T], f32, isOutput=False)
    tvd = nc.declare_dram_parameter("tvals", [1, F_OUT], f16, isOutput=False)
    xsid = nc.declare_dram_parameter("xsinit", [4, T + 1], f16, isOutput=False)
    lensd = nc.declare_dram_parameter("lens", [NB], f32, isOutput=False)
    outd = nc.declare_dram_parameter("out", [NB, F_OUT], f32, isOutput=True)

    with ExitStack() as ctx:
        tc = ctx.enter_context(SplitDrainTileContext(nc))
        const = ctx.enter_context(tc.tile_pool(name="const", bufs=1))
        tmp = ctx.enter_context(tc.tile_pool(name="tmp", bufs=2))
        GA0 = ctx.enter_context(nc.psum_tensor([128, 1, 512], f32))
        GB0 = ctx.enter_context(nc.psum_tensor([128, 1, 512], f32))
        GO0 = ctx.enter_context(nc.psum_tensor([128, 1, 512], f32))
        GA1 = ctx.enter_context(nc.psum_tensor([128, 1, 512], f32))
        GB1 = ctx.enter_context(nc.psum_tensor([128, 1, 512], f32))
        GO1 = ctx.enter_context(nc.psum_tensor([128, 1, 512], f32))
        PB = ctx.enter_context(nc.psum_tensor([128, 1, 512], f32))
        BC = ctx.enter_context(nc.psum_tensor([128, 1, 512], f32))
        G0 = (GA0, GB0, GO0)
        G1 = (GA1, GB1, GO1)
        Pap = PB[0:3, 0, 0:T]              # head output

        _r2slot = {}
        for s, r in enumerate(SLOTS_A):
            _r2slot[r] = (0, s)
        for s, r in enumerate(SLOTS_B):
            _r2slot[r] = (1, s)
        for s, r in enumerate(SLOTS_O):
            _r2slot[r] = (2, s)

        def greg(G, r):
            ti, s = _r2slot[r]
            return G[ti][:, 0, s * T:(s + 1) * T]

        # ---- constants / weights into SBUF ----
        # weight DMAs are chunked and ordered by first use; each chunk's
        # semaphore is absorbed into the PE clock by a tiny ldweights right
        # before its first consuming matmul.
        # DMA ring-flow waits are cumulative per queue and walrus allows
        # only one sync wait per instruction, so the gpsimd queue is
        # reserved for the four output stores; loads alternate between the
        # SP and ACT queues so weight chunks land in parallel and sweeps
        # 0/1 are not DMA-starved (6.4MB of weights vs ~200GB/s per queue).
        queues = (nc.sync.dma_start, nc.scalar.dma_start)
        wx4s = const.tile([4, 2048], f16)
        queues[0](wx4s[:], wx4T[:, :])
        Xs = const.tile([4, T + 1], f16)
        queues[1](Xs[:], xsid[:, :])  # rows 0:3 zero, row 3 = 1 (bias)
        b1rs = const.tile([128, M_TILES * T], f16)
        queues[0](b1rs[:], b1rd[:, :])
        wpss = const.tile([128, 12], f16)
        queues[0](wpss[:], wpT[:, :])
        w1is = const.tile([128, 4 * 2048], f16)
        for qi, (lo, hi) in enumerate(WCHUNKS):
            queues[qi % 2](w1is[:, lo * 512:hi * 512], w1iT[:, lo * 512:hi * 512])
        w0s = const.tile([128, 4 * 2048], f16)
        for qi, (lo, hi) in enumerate(WCHUNKS):
            queues[(qi + 1) % 2](w0s[:, lo * 512:hi * 512], w0T[:, lo * 512:hi * 512])
        w1hs = const.tile([128, 4 * 2048], f16)
        for qi, (lo, hi) in enumerate(WCHUNKS):
            queues[qi % 2](w1hs[:, lo * 512:hi * 512], w1hT[:, lo * 512:hi * 512])
        oh3s = const.tile([3, 3 * 128], f16)
        queues[1](oh3s[:], oh3d[:, :])
        bpcs = const.tile([3, T], f32)
        queues[1](bpcs[:], bpcd[:, :])
        tvs = const.tile([1, F_OUT], f16)
        queues[0](tvs[:], tvd[:, :])
        lenss = const.tile([128, BT], f32)
        queues[1](lenss[:], lensd.rearrange("(m p) -> p m", p=128))

        ones1 = const.tile([1, 128], f16)
        nc.vector.memset(ones1[:], 1.0)

        # states: ping-pong buffers, col 0 = zero state
        H0s = [const.tile([128, 4, T + 1], f16, name=f"h0_{i}") for i in range(2)]
        H1s = [const.tile([128, 4, T + 1], f16, name=f"h1_{i}") for i in range(2)]
        for s in (*H0s, *H1s):
            nc.vector.memset(s[:, :, 0:1], 0.0)

        # DVE absorbers for DVE-consumed const DMAs
        absb = const.tile([1, 5], f32)
        nc.vector.tensor_copy(absb[:, 0:1], b1rs[0:1, 0:1])
        nc.vector.tensor_copy(absb[:, 1:2], bpcs[0:1, 0:1])
        nc.vector.tensor_copy(absb[:, 2:3], tvs[0:1, 0:1])
        nc.vector.tensor_copy(absb[:, 3:4], lenss[0:1, 0:1])
        nc.vector.tensor_copy(absb[:, 4:5], Xs[0:1, 0:1])
        nc.tensor.ldweights(wx4s[0:4, 0:1])

        # broadcast the t-values row across partitions (once)
        tvbc = const.tile([128, F_OUT], f32)
        nc.tensor.matmul(BC[:, 0, F_OUT:2 * F_OUT], lhsT=ones1[:], rhs=tvs[:],
                         start=True, stop=True)
        nc.vector.tensor_copy(tvbc[:], BC[:, 0, F_OUT:2 * F_OUT])

        def cell_elementwise_a(G, cellno, j, bias):
            """Part 1: after tensors A and B are complete -- tanh(g),
            sig(f,i), u, the c scan and tanh(c).  Runs while the o-gate
            matmuls stream through the PE."""
            GA, GB, _ = G
            gba = tmp.tile([128, 6 * T], f16, tag=f"gba{cellno}", bufs=2)
            if bias is not None:
                nc.vector.tensor_add(gba[:], GA[:, 0, 0:6 * T], bias[:, 0:6 * T])
            else:
                nc.vector.tensor_copy(gba[:], GA[:, 0, 0:6 * T])
            tg = tmp.tile([128, 4 * T], f16, tag=f"tg{cellno}_{j}", bufs=1)
            nc.scalar.activation(tg[:], gba[:, 0:4 * T], AF.Tanh)     # tanh(g)
            sgf01 = tmp.tile([128, 2 * T], f16, tag=f"sgf01_{cellno}_{j}", bufs=1)
            nc.scalar.activation(sgf01[:], gba[:, 4 * T:6 * T], AF.Sigmoid)
            gbb = tmp.tile([128, 6 * T], f16, tag=f"gbb{cellno}", bufs=2)
            if bias is not None:
                nc.vector.tensor_add(gbb[:], GB[:, 0, 0:6 * T], bias[:, 6 * T:12 * T])
            else:
                nc.vector.tensor_copy(gbb[:], GB[:, 0, 0:6 * T])
            sgb = tmp.tile([128, 6 * T], f16, tag=f"sgb{cellno}_{j}", bufs=1)
            nc.scalar.activation(sgb[:], gbb[:], AF.Sigmoid)    # sig(f23, i)
            u = tmp.tile([128, 4 * T], f16, tag=f"u{cellno}", bufs=2)
            nc.vector.tensor_mul(u[:], sgb[:, 2 * T:6 * T], tg[:])  # sig(i)*tanh(g)
            cf = tmp.tile([128, 4 * T], f16, tag=f"c{cellno}", bufs=2)
            for c4, sf in ((0, sgf01[:, 0:T]), (1, sgf01[:, T:2 * T]),
                           (2, sgb[:, 0:T]), (3, sgb[:, T:2 * T])):
                nc.vector.tensor_tensor_scan(
                    cf[:, c4 * T:(c4 + 1) * T], sf, u[:, c4 * T:(c4 + 1) * T],
                    0.0, Alu.mult, Alu.add)   # c_t = sig(f_t)*c_{t-1} + u_t
            tcn = tmp.tile([128, 4 * T], f16, tag=f"tc{cellno}_{j}", bufs=1)
            nc.scalar.activation(tcn[:], cf[:], AF.Tanh)
            return tcn

        def cell_elementwise_o(G, tcn, h_out, cellno, j, bias):
            """Part 2: after tensor O -- sig(o), h' = sig(o)*tanh(c)."""
            _, _, GO = G
            gbo = tmp.tile([128, 4 * T], f16, tag=f"gbo{cellno}", bufs=2)
            if bias is not None:
                nc.vector.tensor_add(gbo[:], GO[:, 0, 0:4 * T], bias[:, 12 * T:])
            else:
                nc.vector.tensor_copy(gbo[:], GO[:, 0, 0:4 * T])
            sgo = tmp.tile([128, 4 * T], f16, tag=f"sgo{cellno}_{j}", bufs=1)
            nc.scalar.activation(sgo[:], gbo[:], AF.Sigmoid)
            nc.vector.tensor_mul(h_out, sgo[:], tcn[:])

        def emit_head(h1buf):
            """P(:, t) = W_pc @ h1(t), then Xs[:, 1:] = P + b_pc (fp16)."""
            for k in range(4):
                nc.tensor.matmul(Pap, lhsT=wpss[:, 3 * k:3 * k + 3],
                                 rhs=h1buf[:, k, 1:T + 1],
                                 start=(k == 0), stop=(k == 3))
            nc.vector.tensor_add(Xs[0:3, 1:T + 1], Pap, bpcs[:])

        def absorb(ws, m):
            # absorb the DMA chunk whose first m-tile is m into the PE clock
            nc.tensor.ldweights(ws[:, m * 512:m * 512 + 1])

        for j in range(J_SWEEPS):
            r, w = j % 2, (j + 1) % 2
            if j > 0:
                if j == 1:
                    absorb(wpss, 0)
                emit_head(H1s[r])           # head of sweep j-1 -> x for sweep j
            # G0 = W_hh0 @ h0(prev, shifted) + W_ih0 @ x + b0 (K=4 x+bias
            # pass); one tight accumulation group per gate region.
            for si, m in enumerate(SLOTS):
                if j == 1 and si in (0, 4, 8, 12):
                    absorb(w0s, (m // 4) * 4)
                for k in range(4):
                    if j > 0:
                        nc.tensor.matmul(
                            greg(G0, m),
                            lhsT=w0s[:, m * 512 + k * 128:m * 512 + (k + 1) * 128],
                            rhs=H0s[r][:, k, 0:T],
                            start=(k == 0), stop=False)
                nc.tensor.matmul(
                    greg(G0, m),
                    lhsT=wx4s[0:4, m * 128:(m + 1) * 128],
                    rhs=Xs[0:4, 0:T],
                    start=(j == 0), stop=True)
                if si == len(SLOTS_A) + len(SLOTS_B) - 1:
                    tcn0 = cell_elementwise_a(G0, 0, j, None)
            cell_elementwise_o(G0, tcn0, H0s[w][:, :, 1:T + 1], 0, j, None)
            # G1 = W_ih1 @ h0(this sweep) + W_hh1 @ h1(prev, shifted)
            for si, m in enumerate(SLOTS):
                if j == 0 and si in (0, 4, 8, 12):
                    absorb(w1is, (m // 4) * 4)
                if j == 1 and si in (0, 4, 8, 12):
                    absorb(w1hs, (m // 4) * 4)
                for k in range(4):
                    nc.tensor.matmul(
                        greg(G1, m),
                        lhsT=w1is[:, m * 512 + k * 128:m * 512 + (k + 1) * 128],
                        rhs=H0s[w][:, k, 1:T + 1],
                        start=(k == 0), stop=(j == 0 and k == 3))
                if j > 0:
                    for k in range(4):
                        nc.tensor.matmul(
                            greg(G1, m),
                            lhsT=w1hs[:, m * 512 + k * 128:m * 512 + (k + 1) * 128],
                            rhs=H1s[r][:, k, 0:T],
                            start=False, stop=(k == 3))
                if si == len(SLOTS_A) + len(SLOTS_B) - 1:
                    tcn1 = cell_elementwise_a(G1, 1, j, b1rs)
            cell_elementwise_o(G1, tcn1, H1s[w][:, :, 1:T + 1], 1, j, b1rs)

        # final head -> Xs[0:3, 1:85] = final trajectory points (with bias)
        emit_head(H1s[J_SWEEPS % 2])

        # ---- broadcast + mask + store ----
        nc.tensor.ldweights(oh3s[0:3, 0:1])   # absorb oh3 DMA sem into PE
        for i in range(3):
            nc.tensor.matmul(BC[:, 0, i * T:(i + 1) * T],
                             lhsT=oh3s[0:3, i * 128:(i + 1) * 128],
                             rhs=Xs[0:3, 1:T + 1], start=True, stop=True)
        pq = const.tile([128, T, 3], f32)     # interleaved [t, i] -> col 3t+i
        for i in range(3):
            nc.vector.tensor_copy(pq[:, :, i], BC[:, 0, i * T:(i + 1) * T])
        ot = const.tile([128, BT * F_OUT], f32)
        out_r = outd.rearrange("(n p) f -> p n f", p=128)
        for n in range(BT):
            # out_row = (tvals < len) * p_broadcast, fused in one DVE op
            nc.vector.scalar_tensor_tensor(
                ot[:, n * F_OUT:(n + 1) * F_OUT], tvbc[:], lenss[:, n:n + 1],
                pq[:, :, :], Alu.is_lt, Alu.mult)
            # chunked stores overlap the remaining masking ops, one chunk per
            # DMA queue (finer per-queue chunks would need ring-flow waits on
            # top of the data wait; walrus allows one sync wait each)
            if n % 4 == 3:
                nc.gpsimd.dma_start(
                    out_r[:, n - 3:n + 1, :],
                    ot[:, (n - 3) * F_OUT:(n + 1) * F_OUT])

    return nc


def _prep_inputs(inputs):
    f = lambda k: np.asarray(inputs[k], np.float32)
    Wih0 = _gate_reorder(f("W_ih0"))
    Whh0 = _gate_reorder(f("W_hh0"))
    Wih1 = _gate_reorder(f("W_ih1"))
    Whh1 = _gate_reorder(f("W_hh1"))
    b0 = _gate_reorder(f("b_ih0") + f("b_hh0"))
    b1 = _gate_reorder(f("b_ih1") + f("b_hh1"))
    Wpc = f("W_pc")
    bpc = f("b_pc")

    oh3 = np.zeros((3, 3 * 128), np.float16)
    for i in range(3):
        oh3[i, i * 128:(i + 1) * 128] = 1.0

    # b1 replicated across T in SLOT order: col s*T+t = b1[SLOTS[s]*128+p]
    b1m = b1.reshape(M_TILES, 128)[list(SLOTS)]           # [16 slots, 128]
    b1rep = np.repeat(b1m.T[:, :, None], T, axis=2).reshape(128, M_TILES * T)

    common = {
        "w0T": _lhsT_tiles_mmajor(Whh0.T.copy(), 4).astype(np.float16),
        "w1iT": _lhsT_tiles_mmajor(Wih1.T.copy(), 4).astype(np.float16),
        "w1hT": _lhsT_tiles_mmajor(Whh1.T.copy(), 4).astype(np.float16),
        "wx4T": np.ascontiguousarray(
            np.concatenate([Wih0.T, b0[None, :]], 0)).astype(np.float16),
        "wpT": np.ascontiguousarray(
            Wpc.T.reshape(4, 128, 3).transpose(1, 0, 2).reshape(128, 12)
        ).astype(np.float16),
        "b1rep": np.ascontiguousarray(b1rep).astype(np.float16),
        "oh3": oh3,
        "bpc84": np.ascontiguousarray(np.repeat(bpc[:, None], T, axis=1)),
        "tvals": np.repeat(np.arange(T, dtype=np.float16), IN).reshape(1, F_OUT),
        "xsinit": np.concatenate(
            [np.zeros((3, T + 1), np.float16), np.ones((1, T + 1), np.float16)], 0),
    }
    lens = np.asarray(inputs["seq_lengths"]).astype(np.float32)
    in_maps = []
    for c in range(N_CORES):
        m = dict(common)
        m["lens"] = np.ascontiguousarray(lens[c * NB:(c + 1) * NB])
        in_maps.append(m)
    return in_maps


def kernel(**inputs):
    global _COMPILED, LAST_RESULTS
    from concourse.bass_utils import run_bass_kernel_spmd

    if _COMPILED is None:
        _COMPILED = _build_program()
    nc = _COMPILED

    in_maps = _prep_inputs(inputs)
    res = run_bass_kernel_spmd(nc, in_maps, list(range(N_CORES)))
    LAST_RESULTS = res
    out = np.concatenate([res.results[c]["out"] for c in range(N_CORES)], axis=0)
    return np.ascontiguousarray(out.reshape(B, T, IN))
